# revision 1
# baseline (speedup 1.0000x reference)
"""GNN message passing + global softmax attention + MLP head on 8 TRN2 NeuronCores.

Strategy (node-sharded SPMD, rank enters only via per-core input data):
  - 2 GraphConv layers: aggregation as block-dense adjacency matmul
    aggT[d, dst] = sum_s x_s^T @ A_s with per-core dense count matrix A
    (fp8, exact small ints) kept SBUF-resident across both layers.
  - AllGather h (tiled layout) between layers; AllGather neT before attention.
  - Attention: natural-layout scores S[q, k] on PE, per-q shift from a
    stride-16 subsample max (+margin), exp on ACT with per-partition bias and
    free accum_out for the softmax denominator, xbar DMA-transpose of the
    exp'd tiles, PV matmul accumulating WT[d, q] in PSUM. Mean over q folds
    into a per-q 1/Z scale, one AllReduce of a [128,1] vector.
  - Tiny MLP head replicated on every core (fp32).
"""

import math
import os

import numpy as np
import ml_dtypes

import concourse.bass as bass
import concourse.bacc as bacc
import concourse.tile as tile
from concourse import mybir
from concourse.bass_utils import run_bass_kernel_spmd
from concourse.masks import make_identity

NCORES = 8
NREAL = 10000
NP = 10240           # padded node count
ND = NP // NCORES    # 1280 nodes per core
NT = ND // 128       # 10 q/dst tiles per core
SB = NP // 128       # 80 src blocks
D = 128
KB = 1024            # k block width in attention
KKN = NP // KB       # 10 k blocks
INV = 1.0 / math.sqrt(128.0)
MARGIN = 40.0        # safety margin (scaled units) on the subsample max
KREAL_LAST = NREAL - (KKN - 1) * KB  # 784 valid cols in last k block

BF16 = mybir.dt.bfloat16
FP8 = mybir.dt.float8e4
F32 = mybir.dt.float32

NP_BF16 = mybir.dt.np(BF16)
NP_FP8 = mybir.dt.np(FP8)

_NC_CACHE = {}

RELU = mybir.ActivationFunctionType.Relu
IDENT = mybir.ActivationFunctionType.Identity
EXP = mybir.ActivationFunctionType.Exp
ADD = mybir.AluOpType.add
MULT = mybir.AluOpType.mult
SL3 = ((0, 512), (512, 1024), (1024, 1280))


def _build(phase=9):
    if phase in (51, 52, 53):
        phase_sub, phase = phase, 5
    elif phase in (71, 72, 73):
        phase_sub, phase = phase, 7
    else:
        phase_sub = None
    nc = bacc.Bacc("TRN2", target_bir_lowering=False, debug=False, num_devices=NCORES)

    A_in = nc.dram_tensor("a_cnt", [128, SB, ND], FP8, kind="ExternalInput")
    xt_in = nc.dram_tensor("x_tiled", [128, SB, D], BF16, kind="ExternalInput")
    xTm_in = nc.dram_tensor("xT_mine", [128, ND], BF16, kind="ExternalInput")
    qmask_in = nc.dram_tensor("qmask", [128, NT], F32, kind="ExternalInput")
    names_bf = ["w1r", "w1l", "w2r", "w2l", "wq", "wk", "wv"]
    ins_bf = {n: nc.dram_tensor(n, [D, D], BF16, kind="ExternalInput") for n in names_bf}
    ins_f32 = {
        "b1": nc.dram_tensor("b1", [D, 1], F32, kind="ExternalInput"),
        "b2": nc.dram_tensor("b2", [D, 1], F32, kind="ExternalInput"),
        "qgv": nc.dram_tensor("qgv", [D, 1], F32, kind="ExternalInput"),
        "vgv": nc.dram_tensor("vgv", [D, 1], F32, kind="ExternalInput"),
        "wo": nc.dram_tensor("wo", [D, D], F32, kind="ExternalInput"),
        "wf1": nc.dram_tensor("wf1", [D, 64], F32, kind="ExternalInput"),
        "wf2": nc.dram_tensor("wf2", [64, 32], F32, kind="ExternalInput"),
        "wf3": nc.dram_tensor("wf3", [32, D], F32, kind="ExternalInput"),
        "bo": nc.dram_tensor("bo", [D, 1], F32, kind="ExternalInput"),
        "bf1": nc.dram_tensor("bf1", [64, 1], F32, kind="ExternalInput"),
        "bf2": nc.dram_tensor("bf2", [32, 1], F32, kind="ExternalInput"),
        "bf3": nc.dram_tensor("bf3", [D, 1], F32, kind="ExternalInput"),
    }
    out_t = nc.dram_tensor("out", [1, D], F32, kind="ExternalOutput")
    rg = [list(range(NCORES))]

    with tile.TileContext(nc) as tc:
        with (
            tc.tile_pool(name="dram", bufs=1, space="DRAM") as dram,
            tc.tile_pool(name="const", bufs=1) as cp,
            tc.tile_pool(name="live", bufs=1) as lp,
        ):
            hb_a = dram.tile([128, ND // 2], BF16)
            hb_b = dram.tile([128, ND // 2], BF16)
            hfull_a = dram.tile([NCORES, 128, NT // 2, D], BF16, addr_space="Shared")
            hfull_b = dram.tile([NCORES, 128, NT // 2, D], BF16, addr_space="Shared")
            neb_a = dram.tile([128, ND // 2], BF16)
            neb_b = dram.tile([128, ND // 2], BF16)
            nefull_a = dram.tile([NCORES, 128, ND // 2], BF16, addr_space="Shared")
            nefull_b = dram.tile([NCORES, 128, ND // 2], BF16, addr_space="Shared")
            accb = dram.tile([128, 1], F32)
            accr = dram.tile([128, 1], F32, addr_space="Shared")

            def cload(dram_t, shape, dtype):
                t = cp.tile(shape, dtype, tag=f"c_{dram_t.name}")
                nc.sync.dma_start(out=t[:], in_=dram_t[:])
                return t

            w1r = cload(ins_bf["w1r"], [D, D], BF16)
            w1l = cload(ins_bf["w1l"], [D, D], BF16)
            w2r = cload(ins_bf["w2r"], [D, D], BF16)
            w2l = cload(ins_bf["w2l"], [D, D], BF16)
            wq = cload(ins_bf["wq"], [D, D], BF16)
            wk = cload(ins_bf["wk"], [D, D], BF16)
            wv = cload(ins_bf["wv"], [D, D], BF16)
            b1 = cload(ins_f32["b1"], [D, 1], F32)
            b2 = cload(ins_f32["b2"], [D, 1], F32)
            qgv = cload(ins_f32["qgv"], [D, 1], F32)
            vgv = cload(ins_f32["vgv"], [D, 1], F32)
            wo = cload(ins_f32["wo"], [D, D], F32)
            wf1 = cload(ins_f32["wf1"], [D, 64], F32)
            wf2 = cload(ins_f32["wf2"], [64, 32], F32)
            wf3 = cload(ins_f32["wf3"], [32, D], F32)
            bo = cload(ins_f32["bo"], [D, 1], F32)
            bf1 = cload(ins_f32["bf1"], [64, 1], F32)
            bf2 = cload(ins_f32["bf2"], [32, 1], F32)
            bf3 = cload(ins_f32["bf3"], [D, 1], F32)
            xTm = cload(xTm_in, [128, ND], BF16)
            qmask = cload(qmask_in, [128, NT], F32)
            ident = cp.tile([128, 128], F32)
            make_identity(nc, ident[:])
            ones1 = cp.tile([1, 128], F32)
            nc.gpsimd.memset(ones1[:], 1.0)
            nmarg = cp.tile([128, 1], F32)
            nc.gpsimd.memset(nmarg[:], -MARGIN)

            hT = lp.tile([128, ND], BF16)
            QT = lp.tile([128, ND], BF16)
            neT = lp.tile([128, ND], BF16)

            # ================= message passing =================
            with (
                tc.tile_pool(name="sbL", bufs=1) as sl,
                tc.tile_pool(name="hstp", bufs=4) as hstp,
                tc.tile_pool(name="psL", bufs=1, space="PSUM") as psl,
            ):
                xt = sl.tile([128, SB, D], BF16)
                for j in range(4):
                    nc.sync.dma_start(out=xt[:, 20 * j : 20 * (j + 1), :],
                                      in_=xt_in[:, 20 * j : 20 * (j + 1), :])
                asb = sl.tile([128, SB, ND], FP8)
                for j in range(16):
                    nc.sync.dma_start(
                        out=asb[:, 5 * j : 5 * (j + 1), :],
                        in_=A_in[:, 5 * j : 5 * (j + 1), :],
                    )

                # ----- layer 1 -----
                agg_ps = psl.tile([128, ND], F32, tag="agg", space="PSUM")
                for s in range(SB):
                    for lo, hi in SL3:
                        nc.tensor.matmul(
                            out=agg_ps[:, lo:hi], lhsT=xt[:, s, :],
                            rhs=asb[:, s, lo:hi],
                            start=(s == 0), stop=(s == SB - 1),
                        )
                agg1 = sl.tile([128, ND], BF16, tag="agg1")
                for lo, hi in SL3:
                    nc.vector.tensor_copy(out=agg1[:, lo:hi], in_=agg_ps[:, lo:hi])

                z_ps = psl.tile([128, ND], F32, tag="z", space="PSUM")
                for lo, hi in SL3:
                    nc.tensor.matmul(out=z_ps[:, lo:hi], lhsT=w1r[:],
                                     rhs=xTm[:, lo:hi], start=True, stop=False)
                    nc.tensor.matmul(out=z_ps[:, lo:hi], lhsT=w1l[:],
                                     rhs=agg1[:, lo:hi], start=False, stop=True)
                    nc.scalar.activation(hT[:, lo:hi], z_ps[:, lo:hi], RELU, bias=b1[:])

                if phase >= 2:
                    htiled = sl.tile([128, NT, D], BF16, tag="htiled")
                    nc.sync.dma_start_transpose(out=htiled[:], in_=hT[:])
                    nc.sync.dma_start(
                        out=hb_a[:],
                        in_=htiled[:, 0 : NT // 2, :].rearrange("p t d -> p (t d)"),
                    )
                    nc.sync.dma_start(
                        out=hb_b[:],
                        in_=htiled[:, NT // 2 :, :].rearrange("p t d -> p (t d)"),
                    )
                if phase >= 3:
                    nc.gpsimd.collective_compute(
                        "AllGather", mybir.AluOpType.bypass, replica_groups=rg,
                        ins=[hb_a[:].opt()], outs=[hfull_a[:].opt()],
                    )
                    nc.gpsimd.collective_compute(
                        "AllGather", mybir.AluOpType.bypass, replica_groups=rg,
                        ins=[hb_b[:].opt()], outs=[hfull_b[:].opt()],
                    )

                # ----- layer 2 -----
                if phase >= 4:
                    agg_ps2 = psl.tile([128, ND], F32, tag="agg", space="PSUM")
                    hts_a = sl.tile([128, NCORES, NT // 2, D], BF16, tag="hts_a")
                    hts_b = sl.tile([128, NCORES, NT // 2, D], BF16, tag="hts_b")
                    for c in range(NCORES):
                        nc.sync.dma_start(out=hts_a[:, c, :, :], in_=hfull_a[c])
                    for c in range(NCORES):
                        nc.sync.dma_start(out=hts_b[:, c, :, :], in_=hfull_b[c])
                    sorder = [c * NT + t for t in range(NT) for c in range(NCORES)]
                    for si, s in enumerate(sorder):
                        c, t_loc = s // NT, s % NT
                        hsl = (hts_a if t_loc < NT // 2 else hts_b)[
                            :, c, t_loc % (NT // 2), :
                        ]
                        for lo, hi in SL3:
                            nc.tensor.matmul(
                                out=agg_ps2[:, lo:hi], lhsT=hsl,
                                rhs=asb[:, s, lo:hi],
                                start=(si == 0), stop=(si == SB - 1),
                            )
                    agg2 = sl.tile([128, ND], BF16, tag="agg2")
                    for lo, hi in SL3:
                        nc.vector.tensor_copy(out=agg2[:, lo:hi], in_=agg_ps2[:, lo:hi])

                    z_ps2 = psl.tile([128, ND], F32, tag="z", space="PSUM")
                    for lo, hi in SL3:
                        nc.tensor.matmul(out=z_ps2[:, lo:hi], lhsT=w2r[:],
                                         rhs=hT[:, lo:hi], start=True, stop=False)
                        nc.tensor.matmul(out=z_ps2[:, lo:hi], lhsT=w2l[:],
                                         rhs=agg2[:, lo:hi], start=False, stop=True)
                        nc.scalar.activation(neT[:, lo:hi], z_ps2[:, lo:hi], IDENT,
                                             bias=b2[:])

                    q_ps = psl.tile([128, ND], F32, tag="agg", space="PSUM")
                    for lo, hi in SL3:
                        nc.tensor.matmul(out=q_ps[:, lo:hi], lhsT=wq[:],
                                         rhs=neT[:, lo:hi], start=True, stop=True)
                        nc.vector.tensor_scalar(
                            out=QT[:, lo:hi], in0=q_ps[:, lo:hi],
                            scalar1=qgv[:], scalar2=None, op0=ADD,
                        )

                    nc.sync.dma_start(out=neb_a[:], in_=neT[:, 0 : ND // 2])
                    nc.gpsimd.collective_compute(
                        "AllGather", mybir.AluOpType.bypass, replica_groups=rg,
                        ins=[neb_a[:].opt()], outs=[nefull_a[:].opt()],
                    )
                    nc.sync.dma_start(out=neb_b[:], in_=neT[:, ND // 2 :])
                    nc.gpsimd.collective_compute(
                        "AllGather", mybir.AluOpType.bypass, replica_groups=rg,
                        ins=[neb_b[:].opt()], outs=[nefull_b[:].opt()],
                    )

            # ================= attention =================
            if phase >= 5:
                with (
                    tc.tile_pool(name="sbA", bufs=1) as sa,
                    tc.tile_pool(name="etp", bufs=2) as etp,
                    tc.tile_pool(name="ep", bufs=8) as ep,
                    tc.tile_pool(name="stp", bufs=2, space="PSUM") as stp,
                    tc.tile_pool(name="wtp", bufs=1, space="PSUM") as wtp,
                ):
                    neTf = sa.tile([128, NCORES, ND], BF16)
                    for r in range(NCORES):
                        nc.sync.dma_start(out=neTf[:, r, 0 : ND // 2],
                                          in_=nefull_a[r])
                    for r in range(NCORES):
                        nc.sync.dma_start(out=neTf[:, r, ND // 2 :],
                                          in_=nefull_b[r])
                    neTfl = neTf[:].rearrange("p r j -> p (r j)")

                    KT = sa.tile([128, NP], BF16)
                    for i in range(NP // 512):
                        kps = stp.tile([128, KB], F32, tag="st", space="PSUM")
                        nc.tensor.matmul(out=kps[:, 0:512], lhsT=wk[:],
                                         rhs=neTfl[:, 512 * i : 512 * (i + 1)],
                                         start=True, stop=True)
                        nc.vector.tensor_copy(out=KT[:, 512 * i : 512 * (i + 1)],
                                              in_=kps[:, 0:512])

                    vt = sa.tile([128, SB, D], BF16)
                    for g in range(SB // 8 if phase_sub != 51 else 0):
                        vps = stp.tile([128, KB], F32, tag="st", space="PSUM")
                        for j in range(8):
                            kc = 8 * g + j
                            nc.tensor.matmul(
                                out=vps[:, 128 * j : 128 * (j + 1)],
                                lhsT=neTfl[:, 128 * kc : 128 * (kc + 1)],
                                rhs=wv[:], start=True, stop=True,
                            )
                        nc.vector.tensor_copy(
                            out=vt[:, 8 * g : 8 * (g + 1), :].rearrange(
                                "p s d -> p (s d)"
                            ),
                            in_=vps[:],
                        )

                    KTs = sa.tile([128, 640], BF16)
                    if phase_sub not in (51, 52):
                        nc.vector.tensor_copy(out=KTs[:], in_=KT[:, ::16])
                    negc = sa.tile([128, NT], F32)
                    for t in range(NT if phase_sub not in (51, 52) else 0):
                        sps = stp.tile([128, KB], F32, tag="st", space="PSUM")
                        nc.tensor.matmul(out=sps[:, 0:512],
                                         lhsT=QT[:, 128 * t : 128 * (t + 1)],
                                         rhs=KTs[:, 0:512], start=True, stop=True)
                        nc.tensor.matmul(out=sps[:, 512:640],
                                         lhsT=QT[:, 128 * t : 128 * (t + 1)],
                                         rhs=KTs[:, 512:640], start=True, stop=True)
                        mh = sa.tile([128, 1], F32, tag="mh")
                        nc.vector.reduce_max(mh[:], sps[:, 0:640],
                                             axis=mybir.AxisListType.X)
                        nc.scalar.activation(negc[:, t : t + 1], mh[:], IDENT,
                                             scale=-INV, bias=nmarg[:])

                    WT = wtp.tile([128, ND], F32, tag="wt", space="PSUM")
                    Zbuf = sa.tile([128, NT, KKN], F32)
                    if phase >= 6:
                        def pv_mms(kk, ET2kk):
                            for j in range(KB // 128):
                                kc = (KB // 128) * kk + j
                                for gi, (t0, t1) in enumerate(((0, 4), (4, 8), (8, 10))):
                                    yield (kk, kc, j, gi, t0, t1, ET2kk)

                        def emit_pv_mm(mm):
                            kk, kc, j, gi, t0, t1, ET2kk = mm
                            nc.tensor.matmul(
                                out=WT[:, 128 * t0 : 128 * t1],
                                lhsT=vt[:, kc, :],
                                rhs=ET2kk[:, t0:t1, j, :],
                                start=(kk == 0 and j == 0),
                                stop=(kk == KKN - 1 and j == KB // 128 - 1),
                            )

                        pending_pv = None
                        for kk in range(KKN):
                            ET2 = etp.tile([128, NT, KB // 128, 128], BF16, tag="et2")
                            for t in range(NT):
                                stt = stp.tile([128, KB], F32, tag="st", space="PSUM")
                                for lo, hi in ((0, 512), (512, 1024)):
                                    nc.tensor.matmul(
                                        out=stt[:, lo:hi],
                                        lhsT=QT[:, 128 * t : 128 * (t + 1)],
                                        rhs=KT[:, KB * kk + lo : KB * kk + hi],
                                        start=True, stop=True,
                                    )
                                Et = ep.tile([128, KB], BF16, tag="e")
                                nc.scalar.activation(
                                    Et[:], stt[:], EXP, scale=INV,
                                    bias=negc[:, t : t + 1],
                                )
                                if kk == KKN - 1:
                                    nc.gpsimd.memset(Et[:, KREAL_LAST:], 0)
                                nc.vector.reduce_sum(
                                    Zbuf[:, t, kk : kk + 1],
                                    Et[:, 0 : (KB if kk < KKN - 1 else KREAL_LAST)],
                                    axis=mybir.AxisListType.X,
                                )
                                nc.sync.dma_start_transpose(
                                    out=ET2[:, t, :, :], in_=Et[:]
                                )
                                if pending_pv is not None:
                                    lo2 = (24 * t) // NT
                                    hi2 = (24 * (t + 1)) // NT
                                    for mm in pending_pv[lo2:hi2]:
                                        emit_pv_mm(mm)
                            pending_pv = list(pv_mms(kk, ET2))
                        for mm in pending_pv:
                            emit_pv_mm(mm)

                    if phase >= 7:
                        acc_prev = None
                        junk = sa.tile([128, 128], F32, tag="junk")
                        accs = sa.tile([128, NT], F32)
                        for t in range(NT):
                            zt = sa.tile([128, 1], F32, tag="zt")
                            nc.vector.reduce_sum(zt[:], Zbuf[:, t, :],
                                                 axis=mybir.AxisListType.X)
                            rz = sa.tile([128, 1], F32, tag="rz")
                            nc.vector.reciprocal(rz[:], zt[:])
                            rzm = sa.tile([128, 1], F32, tag="rzm")
                            nc.vector.tensor_tensor(out=rzm[:], in0=rz[:],
                                                    in1=qmask[:, t : t + 1], op=MULT)
                            tp = stp.tile([128, KB], F32, tag="st", space="PSUM")
                            nc.tensor.transpose(out=tp[:1, 0:128], in_=rzm[:],
                                                identity=ident[:])
                            rzrow = sa.tile([1, 128], F32, tag="rzrow")
                            nc.vector.tensor_copy(out=rzrow[:], in_=tp[:1, 0:128])
                            rzb_ps = stp.tile([128, KB], F32, tag="st", space="PSUM")
                            nc.tensor.matmul(out=rzb_ps[:, 0:128], lhsT=ones1[:],
                                             rhs=rzrow[:], start=True, stop=True)
                            rzb = sa.tile([128, 128], F32, tag="rzb")
                            nc.vector.tensor_copy(out=rzb[:], in_=rzb_ps[:, 0:128])
                            nc.vector.tensor_tensor(
                                out=junk[:], in0=WT[:, 128 * t : 128 * (t + 1)],
                                in1=rzb[:], op=MULT,
                            )
                            nc.vector.reduce_sum(accs[:, t : t + 1], junk[:],
                                                 axis=mybir.AxisListType.X)

                        if phase_sub == 71:
                            nc.gpsimd.dma_start(out=out_t[:, 0:10], in_=accs[:1, :])
                        accsb = sa.tile([128, 1], F32, tag="accsb")
                        nc.vector.reduce_sum(accsb[:], accs[:],
                                             axis=mybir.AxisListType.X)
                        nc.sync.dma_start(out=accb[:], in_=accsb[:])
                        nc.gpsimd.collective_compute(
                            "AllReduce", ADD, replica_groups=rg,
                            ins=[accb[:].opt()], outs=[accr[:].opt()],
                        )
                        if phase_sub != 71:
                            if phase_sub == 72:
                                nc.gpsimd.dma_start(out=out_t[:, 0:1], in_=accsb[:1, :])
                            if phase_sub != 72:
                                accg = sa.tile([128, 1], F32, tag="accg")
                                nc.sync.dma_start(out=accg[:], in_=accr[:])
                                aggc = sa.tile([128, 1], F32, tag="aggc")
                                nc.scalar.activation(aggc[:], accg[:], IDENT,
                                                     scale=1.0 / NREAL, bias=vgv[:])

                                hps = stp.tile([128, KB], F32, tag="st", space="PSUM")
                                nc.tensor.matmul(out=hps[:, 0:1], lhsT=wo[:], rhs=aggc[:],
                                                 start=True, stop=True)
                                state = sa.tile([128, 1], F32, tag="state")
                                nc.scalar.activation(state[:], hps[:, 0:1], IDENT, bias=bo[:])
                                hps2 = stp.tile([128, KB], F32, tag="st", space="PSUM")
                                nc.tensor.matmul(out=hps2[:64, 0:1], lhsT=wf1[:], rhs=state[:],
                                                 start=True, stop=True)
                                x1 = sa.tile([64, 1], F32, tag="x1")
                                nc.scalar.activation(x1[:], hps2[:64, 0:1], RELU, bias=bf1[:])
                                hps3 = stp.tile([128, KB], F32, tag="st", space="PSUM")
                                nc.tensor.matmul(out=hps3[:32, 0:1], lhsT=wf2[:], rhs=x1[:],
                                                 start=True, stop=True)
                                x2 = sa.tile([32, 1], F32, tag="x2")
                                nc.scalar.activation(x2[:], hps3[:32, 0:1], RELU, bias=bf2[:])
                                hps4 = stp.tile([128, KB], F32, tag="st", space="PSUM")
                                nc.tensor.matmul(out=hps4[:, 0:1], lhsT=wf3[:], rhs=x2[:],
                                                 start=True, stop=True)
                                lg = sa.tile([128, 1], F32, tag="lg")
                                nc.scalar.activation(lg[:], hps4[:, 0:1], IDENT, bias=bf3[:])
                                if phase_sub == 73:
                                    nc.gpsimd.dma_start(out=out_t[:, 0:1], in_=lg[:1, :])
                                if phase_sub != 73:
                                    tps = stp.tile([128, KB], F32, tag="st", space="PSUM")
                                    nc.tensor.transpose(out=tps[:1, 0:128], in_=lg[:], identity=ident[:])
                                    er = sa.tile([1, 128], F32, tag="er")
                                    zf = sa.tile([1, 1], F32, tag="zf")
                                    nc.scalar.activation(er[:], tps[:1, 0:128], EXP, accum_out=zf[:])
                                    rzf = sa.tile([1, 1], F32, tag="rzf")
                                    nc.vector.reciprocal(rzf[:], zf[:])
                                    orow = sa.tile([1, 128], F32, tag="orow")
                                    nc.vector.tensor_scalar(out=orow[:], in0=er[:], scalar1=rzf[:],
                                                            scalar2=None, op0=MULT)
                                    nc.sync.dma_start(out=out_t[:], in_=orow[:])



                    elif phase == 6:
                        nc.gpsimd.dma_start(out=out_t[:, 0:100], in_=Zbuf[:1, :, :].rearrange("p t k -> p (t k)"))
                    else:
                        nc.gpsimd.dma_start(out=out_t[:], in_=KT[:1, 0:128])
            else:
                src = {1: hT, 2: hT, 3: hT, 4: neT}[phase]
                nc.gpsimd.dma_start(out=out_t[:], in_=src[:1, 0:128])

    nc.compile()
    return nc


def _get_nc():
    phase = int(os.environ.get("K_PHASE", "9"))
    key = ("nc", phase)
    if key not in _NC_CACHE:
        _NC_CACHE[key] = _build(phase)
    return _NC_CACHE[key]


def _prep_in_maps(inputs):
    f32 = np.float32
    x = np.asarray(inputs["node_features"], f32)
    g = np.asarray(inputs["global_info"], f32)
    ei = np.asarray(inputs["edge_index"])
    src = np.asarray(ei[0], np.int64)
    dst = np.asarray(ei[1], np.int64)

    xp = np.zeros((NP, D), f32)
    xp[:NREAL] = x
    xb = xp.astype(NP_BF16)
    x_tiled = np.ascontiguousarray(xb.reshape(SB, 128, D).transpose(1, 0, 2))

    qgv = (np.asarray(inputs["bQ"], f32)
           + (g @ np.asarray(inputs["WQg"], f32))[0]
           + np.asarray(inputs["bQg"], f32)).reshape(D, 1)
    vgv = (np.asarray(inputs["bV"], f32)
           + (g @ np.asarray(inputs["WVg"], f32))[0]
           + np.asarray(inputs["bVg"], f32)).reshape(D, 1)

    def bf(name):
        return np.ascontiguousarray(np.asarray(inputs[name], f32).astype(NP_BF16))

    shared = {
        "w1r": bf("W1_root"), "w1l": bf("W1_rel"),
        "w2r": bf("W2_root"), "w2l": bf("W2_rel"),
        "wq": bf("WQ"), "wk": bf("WK"), "wv": bf("WV"),
        "b1": np.asarray(inputs["b1"], f32).reshape(D, 1),
        "b2": np.asarray(inputs["b2"], f32).reshape(D, 1),
        "qgv": qgv, "vgv": vgv,
        "wo": np.asarray(inputs["Wo"], f32),
        "wf1": np.asarray(inputs["Wfc1"], f32),
        "wf2": np.asarray(inputs["Wfc2"], f32),
        "wf3": np.asarray(inputs["Wfc3"], f32),
        "bo": np.asarray(inputs["bo"], f32).reshape(D, 1),
        "bf1": np.asarray(inputs["bfc1"], f32).reshape(64, 1),
        "bf2": np.asarray(inputs["bfc2"], f32).reshape(32, 1),
        "bf3": np.asarray(inputs["bfc3"], f32).reshape(D, 1),
        "x_tiled": x_tiled,
    }

    core_of = dst // ND
    in_maps = []
    nodes = np.arange(NP)
    for c in range(NCORES):
        m = core_of == c
        A = np.zeros((NP, ND), f32)
        np.add.at(A, (src[m], dst[m] - ND * c), 1.0)
        Ac = np.ascontiguousarray(
            A.reshape(SB, 128, ND).transpose(1, 0, 2)
        ).astype(NP_FP8)
        xTm = np.ascontiguousarray(xb[ND * c : ND * (c + 1)].T)
        qm = (nodes[ND * c : ND * (c + 1)] < NREAL).astype(f32)
        qmask = np.ascontiguousarray(qm.reshape(NT, 128).T)
        in_maps.append({**shared, "a_cnt": Ac, "xT_mine": xTm, "qmask": qmask})
    return in_maps


def kernel(**inputs):
    nc = _get_nc()
    in_maps = _prep_in_maps(inputs)
    res = run_bass_kernel_spmd(nc, in_maps, core_ids=list(range(NCORES)))
    return np.asarray(res.results[0]["out"], np.float32)



# revision 3
# speedup vs baseline: 1.0775x; 1.0775x over previous
"""GNN message passing + global softmax attention + MLP head on 8 TRN2 NeuronCores.

Strategy (node-sharded SPMD, rank enters only via per-core input data):
  - 2 GraphConv layers: aggregation as block-dense adjacency matmul
    aggT[d, dst] = sum_s x_s^T @ A_s with per-core dense count matrix A
    (fp8, exact small ints) kept SBUF-resident across both layers.
  - AllGather h (tiled layout) between layers; AllGather neT before attention.
  - Attention exploits the final mean-pool over queries:
      mean_q softmax(S)_q V = (1/N) (sum_q P[q,:]) V = (1/N) w^T V
      w^T V = (w^T ne) Wv + (sum w) * vgv,  sum w = N exactly.
    So per q-tile row: scores S[q,k] on PE, exp on ACT with per-partition
    shift bias and free accum_out giving the row sums Z, then a fused DVE
    scalar_tensor_tensor accumulates P = E/Z into Pacc (bf16). A ones-matmul
    reduces Pacc over q into w, one matvec w^T ne gives a [128,1] vector,
    one AllReduce, then Wv / MLP head replicated on every core (fp32).
"""

import math
import os

import numpy as np
import ml_dtypes

import concourse.bass as bass
import concourse.bacc as bacc
import concourse.tile as tile
from concourse import mybir
from concourse.bass_utils import run_bass_kernel_spmd
from concourse.masks import make_identity

NCORES = 8
NREAL = 10000
NP = 10240           # padded node count
ND = NP // NCORES    # 1280 nodes per core
NT = ND // 128       # 10 q/dst tiles per core
SB = NP // 128       # 80 src blocks
D = 128
PW = 2048            # exp/score pair width (2 PSUM banks x 2)
NPAIR = NP // PW     # 5 score pairs per q tile
INV = 1.0 / math.sqrt(128.0)
MARGIN = 40.0        # safety margin (scaled units) on the subsample max
PAIR_LAST = NREAL - (NPAIR - 1) * PW  # 1808 valid cols in last pair

BF16 = mybir.dt.bfloat16
FP8 = mybir.dt.float8e4
F32 = mybir.dt.float32

NP_BF16 = mybir.dt.np(BF16)
NP_FP8 = mybir.dt.np(FP8)

_NC_CACHE = {}

RELU = mybir.ActivationFunctionType.Relu
IDENT = mybir.ActivationFunctionType.Identity
EXP = mybir.ActivationFunctionType.Exp
ADD = mybir.AluOpType.add
MULT = mybir.AluOpType.mult
AXX = mybir.AxisListType.X
SL3 = ((0, 512), (512, 1024), (1024, 1280))


def _build():
    nc = bacc.Bacc("TRN2", target_bir_lowering=False, debug=False, num_devices=NCORES)

    A_in = nc.dram_tensor("a_cnt", [128, SB, ND], FP8, kind="ExternalInput")
    xt_in = nc.dram_tensor("x_tiled", [128, SB, D], BF16, kind="ExternalInput")
    xTm_in = nc.dram_tensor("xT_mine", [128, ND], BF16, kind="ExternalInput")
    qmask_in = nc.dram_tensor("qmask", [128, NT], F32, kind="ExternalInput")
    names_bf = ["w1r", "w1l", "w2r", "w2l", "wq", "wk"]
    ins_bf = {n: nc.dram_tensor(n, [D, D], BF16, kind="ExternalInput") for n in names_bf}
    ins_f32 = {
        "b1": nc.dram_tensor("b1", [D, 1], F32, kind="ExternalInput"),
        "b2": nc.dram_tensor("b2", [D, 1], F32, kind="ExternalInput"),
        "qgv": nc.dram_tensor("qgv", [D, 1], F32, kind="ExternalInput"),
        "vgv": nc.dram_tensor("vgv", [D, 1], F32, kind="ExternalInput"),
        "wv32": nc.dram_tensor("wv32", [D, D], F32, kind="ExternalInput"),
        "wo": nc.dram_tensor("wo", [D, D], F32, kind="ExternalInput"),
        "wf1": nc.dram_tensor("wf1", [D, 64], F32, kind="ExternalInput"),
        "wf2": nc.dram_tensor("wf2", [64, 32], F32, kind="ExternalInput"),
        "wf3": nc.dram_tensor("wf3", [32, D], F32, kind="ExternalInput"),
        "bo": nc.dram_tensor("bo", [D, 1], F32, kind="ExternalInput"),
        "bf1": nc.dram_tensor("bf1", [64, 1], F32, kind="ExternalInput"),
        "bf2": nc.dram_tensor("bf2", [32, 1], F32, kind="ExternalInput"),
        "bf3": nc.dram_tensor("bf3", [D, 1], F32, kind="ExternalInput"),
    }
    out_t = nc.dram_tensor("out", [1, D], F32, kind="ExternalOutput")
    rg = [list(range(NCORES))]

    with tile.TileContext(nc) as tc:
        with (
            tc.tile_pool(name="dram", bufs=1, space="DRAM") as dram,
            tc.tile_pool(name="const", bufs=1) as cp,
            tc.tile_pool(name="live", bufs=1) as lp,
        ):
            hb_a = dram.tile([128, ND // 2], BF16)
            hb_b = dram.tile([128, ND // 2], BF16)
            hfull_a = dram.tile([NCORES, 128, NT // 2, D], BF16, addr_space="Shared")
            hfull_b = dram.tile([NCORES, 128, NT // 2, D], BF16, addr_space="Shared")
            neb_a = dram.tile([128, ND // 2], BF16)
            neb_b = dram.tile([128, ND // 2], BF16)
            nefull_a = dram.tile([NCORES, 128, ND // 2], BF16, addr_space="Shared")
            nefull_b = dram.tile([NCORES, 128, ND // 2], BF16, addr_space="Shared")
            accb = dram.tile([128, 1], F32)
            accr = dram.tile([128, 1], F32, addr_space="Shared")

            def cload(dram_t, shape, dtype):
                t = cp.tile(shape, dtype, tag=f"c_{dram_t.name}")
                nc.sync.dma_start(out=t[:], in_=dram_t[:])
                return t

            w1r = cload(ins_bf["w1r"], [D, D], BF16)
            w1l = cload(ins_bf["w1l"], [D, D], BF16)
            w2r = cload(ins_bf["w2r"], [D, D], BF16)
            w2l = cload(ins_bf["w2l"], [D, D], BF16)
            wq = cload(ins_bf["wq"], [D, D], BF16)
            wk = cload(ins_bf["wk"], [D, D], BF16)
            b1 = cload(ins_f32["b1"], [D, 1], F32)
            b2 = cload(ins_f32["b2"], [D, 1], F32)
            qgv = cload(ins_f32["qgv"], [D, 1], F32)
            vgv = cload(ins_f32["vgv"], [D, 1], F32)
            wv32 = cload(ins_f32["wv32"], [D, D], F32)
            wo = cload(ins_f32["wo"], [D, D], F32)
            wf1 = cload(ins_f32["wf1"], [D, 64], F32)
            wf2 = cload(ins_f32["wf2"], [64, 32], F32)
            wf3 = cload(ins_f32["wf3"], [32, D], F32)
            bo = cload(ins_f32["bo"], [D, 1], F32)
            bf1 = cload(ins_f32["bf1"], [64, 1], F32)
            bf2 = cload(ins_f32["bf2"], [32, 1], F32)
            bf3 = cload(ins_f32["bf3"], [D, 1], F32)
            xTm = cload(xTm_in, [128, ND], BF16)
            qmask = cload(qmask_in, [128, NT], F32)
            ident = cp.tile([128, 128], F32)
            make_identity(nc, ident[:])
            ones_bf = cp.tile([128, 1], BF16)
            nc.gpsimd.memset(ones_bf[:], 1.0)
            nmarg = cp.tile([128, 1], F32)
            nc.gpsimd.memset(nmarg[:], -MARGIN)

            hT = lp.tile([128, ND], BF16)
            QT = lp.tile([128, ND], BF16)
            neT = lp.tile([128, ND], BF16)

            # ================= message passing =================
            with (
                tc.tile_pool(name="sbL", bufs=1) as sl,
                tc.tile_pool(name="hstp", bufs=4) as hstp,
                tc.tile_pool(name="psL", bufs=1, space="PSUM") as psl,
            ):
                xt = sl.tile([128, SB, D], BF16)
                for j in range(4):
                    nc.sync.dma_start(out=xt[:, 20 * j : 20 * (j + 1), :],
                                      in_=xt_in[:, 20 * j : 20 * (j + 1), :])
                asb = sl.tile([128, SB, ND], FP8)
                for j in range(16):
                    nc.sync.dma_start(
                        out=asb[:, 5 * j : 5 * (j + 1), :],
                        in_=A_in[:, 5 * j : 5 * (j + 1), :],
                    )

                # ----- layer 1 -----
                agg_ps = psl.tile([128, ND], F32, tag="agg", space="PSUM")
                for s in range(SB):
                    for lo, hi in SL3:
                        nc.tensor.matmul(
                            out=agg_ps[:, lo:hi], lhsT=xt[:, s, :],
                            rhs=asb[:, s, lo:hi],
                            start=(s == 0), stop=(s == SB - 1),
                        )
                agg1 = sl.tile([128, ND], BF16, tag="agg1")
                for lo, hi in SL3:
                    nc.vector.tensor_copy(out=agg1[:, lo:hi], in_=agg_ps[:, lo:hi])

                z_ps = psl.tile([128, ND], F32, tag="z", space="PSUM")
                for lo, hi in SL3:
                    nc.tensor.matmul(out=z_ps[:, lo:hi], lhsT=w1r[:],
                                     rhs=xTm[:, lo:hi], start=True, stop=False)
                    nc.tensor.matmul(out=z_ps[:, lo:hi], lhsT=w1l[:],
                                     rhs=agg1[:, lo:hi], start=False, stop=True)
                    nc.scalar.activation(hT[:, lo:hi], z_ps[:, lo:hi], RELU, bias=b1[:])

                htiled = sl.tile([128, NT, D], BF16, tag="htiled")
                nc.sync.dma_start_transpose(out=htiled[:], in_=hT[:])
                nc.sync.dma_start(
                    out=hb_a[:],
                    in_=htiled[:, 0 : NT // 2, :].rearrange("p t d -> p (t d)"),
                )
                nc.sync.dma_start(
                    out=hb_b[:],
                    in_=htiled[:, NT // 2 :, :].rearrange("p t d -> p (t d)"),
                )
                nc.gpsimd.collective_compute(
                    "AllGather", mybir.AluOpType.bypass, replica_groups=rg,
                    ins=[hb_a[:].opt()], outs=[hfull_a[:].opt()],
                )
                nc.gpsimd.collective_compute(
                    "AllGather", mybir.AluOpType.bypass, replica_groups=rg,
                    ins=[hb_b[:].opt()], outs=[hfull_b[:].opt()],
                )

                # ----- layer 2 -----
                agg_ps2 = psl.tile([128, ND], F32, tag="agg", space="PSUM")
                hts_a = sl.tile([128, NCORES, NT // 2, D], BF16, tag="hts_a")
                hts_b = sl.tile([128, NCORES, NT // 2, D], BF16, tag="hts_b")
                for c in range(NCORES):
                    nc.sync.dma_start(out=hts_a[:, c, :, :], in_=hfull_a[c])
                for c in range(NCORES):
                    nc.sync.dma_start(out=hts_b[:, c, :, :], in_=hfull_b[c])
                sorder = [c * NT + t for t in range(NT) for c in range(NCORES)]
                for si, s in enumerate(sorder):
                    c, t_loc = s // NT, s % NT
                    hsl = (hts_a if t_loc < NT // 2 else hts_b)[
                        :, c, t_loc % (NT // 2), :
                    ]
                    for lo, hi in SL3:
                        nc.tensor.matmul(
                            out=agg_ps2[:, lo:hi], lhsT=hsl,
                            rhs=asb[:, s, lo:hi],
                            start=(si == 0), stop=(si == SB - 1),
                        )
                agg2 = sl.tile([128, ND], BF16, tag="agg2")
                for lo, hi in SL3:
                    nc.vector.tensor_copy(out=agg2[:, lo:hi], in_=agg_ps2[:, lo:hi])

                z_ps2 = psl.tile([128, ND], F32, tag="z", space="PSUM")
                for lo, hi in SL3:
                    nc.tensor.matmul(out=z_ps2[:, lo:hi], lhsT=w2r[:],
                                     rhs=hT[:, lo:hi], start=True, stop=False)
                    nc.tensor.matmul(out=z_ps2[:, lo:hi], lhsT=w2l[:],
                                     rhs=agg2[:, lo:hi], start=False, stop=True)
                    nc.scalar.activation(neT[:, lo:hi], z_ps2[:, lo:hi], IDENT,
                                         bias=b2[:])

                q_ps = psl.tile([128, ND], F32, tag="agg", space="PSUM")
                for lo, hi in SL3:
                    nc.tensor.matmul(out=q_ps[:, lo:hi], lhsT=wq[:],
                                     rhs=neT[:, lo:hi], start=True, stop=True)
                    nc.vector.tensor_scalar(
                        out=QT[:, lo:hi], in0=q_ps[:, lo:hi],
                        scalar1=qgv[:], scalar2=None, op0=ADD,
                    )

                nc.sync.dma_start(out=neb_a[:], in_=neT[:, 0 : ND // 2])
                nc.gpsimd.collective_compute(
                    "AllGather", mybir.AluOpType.bypass, replica_groups=rg,
                    ins=[neb_a[:].opt()], outs=[nefull_a[:].opt()],
                )
                nc.sync.dma_start(out=neb_b[:], in_=neT[:, ND // 2 :])
                nc.gpsimd.collective_compute(
                    "AllGather", mybir.AluOpType.bypass, replica_groups=rg,
                    ins=[neb_b[:].opt()], outs=[nefull_b[:].opt()],
                )

            # ================= attention =================
            with (
                tc.tile_pool(name="sbA", bufs=1) as sa,
                tc.tile_pool(name="etp", bufs=2) as etp,
                tc.tile_pool(name="stp", bufs=2, space="PSUM") as stp,
            ):
                neTf = sa.tile([128, NCORES, ND], BF16)
                for r in range(NCORES):
                    nc.sync.dma_start(out=neTf[:, r, 0 : ND // 2],
                                      in_=nefull_a[r])
                for r in range(NCORES):
                    nc.sync.dma_start(out=neTf[:, r, ND // 2 :],
                                      in_=nefull_b[r])
                neTfl = neTf[:].rearrange("p r j -> p (r j)")

                # ne in node-tiled layout for the final w^T ne matvec
                net_t = sa.tile([128, SB, D], BF16)
                nc.sync.dma_start_transpose(out=net_t[:], in_=neTfl)

                KT = sa.tile([128, NP], BF16)
                for i in range(NP // PW):
                    kps = stp.tile([128, PW], F32, tag="st", space="PSUM")
                    for j in range(4):
                        nc.tensor.matmul(
                            out=kps[:, 512 * j : 512 * (j + 1)], lhsT=wk[:],
                            rhs=neTfl[:, PW * i + 512 * j : PW * i + 512 * (j + 1)],
                            start=True, stop=True,
                        )
                    nc.vector.tensor_copy(out=KT[:, PW * i : PW * (i + 1)],
                                          in_=kps[:])

                # per-row shift from stride-16 subsample max (+margin)
                KTs = sa.tile([128, 640], BF16)
                nc.vector.tensor_copy(out=KTs[:], in_=KT[:, ::16])
                negc = sa.tile([128, NT], F32)
                for t in range(NT):
                    sps = stp.tile([128, PW], F32, tag="st", space="PSUM")
                    nc.tensor.matmul(out=sps[:, 0:512],
                                     lhsT=QT[:, 128 * t : 128 * (t + 1)],
                                     rhs=KTs[:, 0:512], start=True, stop=True)
                    nc.tensor.matmul(out=sps[:, 512:640],
                                     lhsT=QT[:, 128 * t : 128 * (t + 1)],
                                     rhs=KTs[:, 512:640], start=True, stop=True)
                    mh = sa.tile([128, 1], F32, tag="mh")
                    nc.vector.reduce_max(mh[:], sps[:, 0:640], axis=AXX)
                    nc.scalar.activation(negc[:, t : t + 1], mh[:], IDENT,
                                         scale=-INV, bias=nmarg[:])

                # main loop: scores -> exp(+Z) -> P accumulation
                Zbuf = sa.tile([128, NT, NPAIR], F32)
                Pacc = [sa.tile([128, PW], BF16, tag=f"pacc{pp}",
                                name=f"pacc{pp}")
                        for pp in range(NPAIR)]
                for t in range(NT):
                    ets = []
                    for pp in range(NPAIR):
                        stt = stp.tile([128, PW], F32, tag="st", space="PSUM")
                        for j in range(4):
                            nc.tensor.matmul(
                                out=stt[:, 512 * j : 512 * (j + 1)],
                                lhsT=QT[:, 128 * t : 128 * (t + 1)],
                                rhs=KT[:, PW * pp + 512 * j : PW * pp + 512 * (j + 1)],
                                start=True, stop=True,
                            )
                        Et = etp.tile([128, PW], BF16, tag=f"e{pp}")
                        if pp < NPAIR - 1:
                            nc.scalar.activation(
                                Et[:], stt[:], EXP, scale=INV,
                                bias=negc[:, t : t + 1],
                                accum_out=Zbuf[:, t, pp : pp + 1],
                            )
                        else:
                            nc.scalar.activation(
                                Et[:], stt[:], EXP, scale=INV,
                                bias=negc[:, t : t + 1],
                            )
                            nc.gpsimd.memset(Et[:, PAIR_LAST:], 0)
                            nc.vector.reduce_sum(Zbuf[:, t, pp : pp + 1],
                                                 Et[:], axis=AXX)
                        ets.append(Et)
                    zt = sa.tile([128, 1], F32, tag=f"zt{t % 2}")
                    nc.vector.reduce_sum(zt[:], Zbuf[:, t, :], axis=AXX)
                    rz = sa.tile([128, 1], F32, tag=f"rz{t % 2}")
                    nc.vector.reciprocal(rz[:], zt[:])
                    rzm = sa.tile([128, 1], F32, tag=f"rzm{t % 2}")
                    nc.vector.tensor_tensor(out=rzm[:], in0=rz[:],
                                            in1=qmask[:, t : t + 1], op=MULT)
                    for pp in range(NPAIR):
                        if t == 0:
                            nc.vector.tensor_scalar(
                                out=Pacc[pp][:], in0=ets[pp][:],
                                scalar1=rzm[:], scalar2=None, op0=MULT,
                            )
                        else:
                            nc.vector.scalar_tensor_tensor(
                                out=Pacc[pp][:], in0=ets[pp][:], scalar=rzm[:],
                                in1=Pacc[pp][:], op0=MULT, op1=ADD,
                            )

                # w[k] = sum_q Pacc[q, k] via ones-matmuls, one column per k-block
                w_ps = stp.tile([128, PW], F32, tag="st", space="PSUM")
                for pp in range(NPAIR):
                    for j in range(PW // 128):
                        idx = (PW // 128) * pp + j
                        nc.tensor.matmul(
                            out=w_ps[:, idx : idx + 1],
                            lhsT=Pacc[pp][:, 128 * j : 128 * (j + 1)],
                            rhs=ones_bf[:], start=True, stop=True,
                        )
                wsb = sa.tile([128, SB], BF16)
                nc.vector.tensor_copy(out=wsb[:], in_=w_ps[:, 0:SB])

                # out_col[d] = sum_k w[k] ne[k, d]
                oc_ps = stp.tile([128, PW], F32, tag="st", space="PSUM")
                for c in range(SB):
                    nc.tensor.matmul(out=oc_ps[:, 0:1], lhsT=net_t[:, c, :],
                                     rhs=wsb[:, c : c + 1],
                                     start=(c == 0), stop=(c == SB - 1))
                accsb = sa.tile([128, 1], F32, tag="accsb")
                nc.vector.tensor_copy(out=accsb[:], in_=oc_ps[:, 0:1])
                nc.sync.dma_start(out=accb[:], in_=accsb[:])
                nc.gpsimd.collective_compute(
                    "AllReduce", ADD, replica_groups=rg,
                    ins=[accb[:].opt()], outs=[accr[:].opt()],
                )
                accg = sa.tile([128, 1], F32, tag="accg")
                nc.sync.dma_start(out=accg[:], in_=accr[:])

                # aggregated = (1/N) Wv^T out_col + vgv, then MLP head
                vps = stp.tile([128, PW], F32, tag="st", space="PSUM")
                nc.tensor.matmul(out=vps[:, 0:1], lhsT=wv32[:], rhs=accg[:],
                                 start=True, stop=True)
                vagg = sa.tile([128, 1], F32, tag="vagg")
                nc.scalar.activation(vagg[:], vps[:, 0:1], IDENT,
                                     scale=1.0 / NREAL, bias=vgv[:])

                hps = stp.tile([128, PW], F32, tag="st", space="PSUM")
                nc.tensor.matmul(out=hps[:, 0:1], lhsT=wo[:], rhs=vagg[:],
                                 start=True, stop=True)
                state = sa.tile([128, 1], F32, tag="state")
                nc.scalar.activation(state[:], hps[:, 0:1], IDENT, bias=bo[:])
                hps2 = stp.tile([128, PW], F32, tag="st", space="PSUM")
                nc.tensor.matmul(out=hps2[:64, 0:1], lhsT=wf1[:], rhs=state[:],
                                 start=True, stop=True)
                x1 = sa.tile([64, 1], F32, tag="x1")
                nc.scalar.activation(x1[:], hps2[:64, 0:1], RELU, bias=bf1[:])
                hps3 = stp.tile([128, PW], F32, tag="st", space="PSUM")
                nc.tensor.matmul(out=hps3[:32, 0:1], lhsT=wf2[:], rhs=x1[:],
                                 start=True, stop=True)
                x2 = sa.tile([32, 1], F32, tag="x2")
                nc.scalar.activation(x2[:], hps3[:32, 0:1], RELU, bias=bf2[:])
                hps4 = stp.tile([128, PW], F32, tag="st", space="PSUM")
                nc.tensor.matmul(out=hps4[:, 0:1], lhsT=wf3[:], rhs=x2[:],
                                 start=True, stop=True)
                lg = sa.tile([128, 1], F32, tag="lg")
                nc.scalar.activation(lg[:], hps4[:, 0:1], IDENT, bias=bf3[:])

                tps = stp.tile([128, PW], F32, tag="st", space="PSUM")
                nc.tensor.transpose(out=tps[:1, 0:128], in_=lg[:],
                                    identity=ident[:])
                er = sa.tile([1, 128], F32, tag="er")
                zf = sa.tile([1, 1], F32, tag="zf")
                nc.scalar.activation(er[:], tps[:1, 0:128], EXP, accum_out=zf[:])
                rzf = sa.tile([1, 1], F32, tag="rzf")
                nc.vector.reciprocal(rzf[:], zf[:])
                orow = sa.tile([1, 128], F32, tag="orow")
                nc.vector.tensor_scalar(out=orow[:], in0=er[:], scalar1=rzf[:],
                                        scalar2=None, op0=MULT)
                nc.sync.dma_start(out=out_t[:], in_=orow[:])

    nc.compile()
    return nc


def _get_nc():
    if "nc" not in _NC_CACHE:
        _NC_CACHE["nc"] = _build()
    return _NC_CACHE["nc"]


def _prep_in_maps(inputs):
    f32 = np.float32
    x = np.asarray(inputs["node_features"], f32)
    g = np.asarray(inputs["global_info"], f32)
    ei = np.asarray(inputs["edge_index"])
    src = np.asarray(ei[0], np.int64)
    dst = np.asarray(ei[1], np.int64)

    xp = np.zeros((NP, D), f32)
    xp[:NREAL] = x
    xb = xp.astype(NP_BF16)
    x_tiled = np.ascontiguousarray(xb.reshape(SB, 128, D).transpose(1, 0, 2))

    qgv = (np.asarray(inputs["bQ"], f32)
           + (g @ np.asarray(inputs["WQg"], f32))[0]
           + np.asarray(inputs["bQg"], f32)).reshape(D, 1)
    vgv = (np.asarray(inputs["bV"], f32)
           + (g @ np.asarray(inputs["WVg"], f32))[0]
           + np.asarray(inputs["bVg"], f32)).reshape(D, 1)

    def bf(name):
        return np.ascontiguousarray(np.asarray(inputs[name], f32).astype(NP_BF16))

    shared = {
        "w1r": bf("W1_root"), "w1l": bf("W1_rel"),
        "w2r": bf("W2_root"), "w2l": bf("W2_rel"),
        "wq": bf("WQ"), "wk": bf("WK"),
        "b1": np.asarray(inputs["b1"], f32).reshape(D, 1),
        "b2": np.asarray(inputs["b2"], f32).reshape(D, 1),
        "qgv": qgv, "vgv": vgv,
        "wv32": np.asarray(inputs["WV"], f32),
        "wo": np.asarray(inputs["Wo"], f32),
        "wf1": np.asarray(inputs["Wfc1"], f32),
        "wf2": np.asarray(inputs["Wfc2"], f32),
        "wf3": np.asarray(inputs["Wfc3"], f32),
        "bo": np.asarray(inputs["bo"], f32).reshape(D, 1),
        "bf1": np.asarray(inputs["bfc1"], f32).reshape(64, 1),
        "bf2": np.asarray(inputs["bfc2"], f32).reshape(32, 1),
        "bf3": np.asarray(inputs["bfc3"], f32).reshape(D, 1),
        "x_tiled": x_tiled,
    }

    core_of = dst // ND
    in_maps = []
    nodes = np.arange(NP)
    for c in range(NCORES):
        m = core_of == c
        A = np.zeros((NP, ND), f32)
        np.add.at(A, (src[m], dst[m] - ND * c), 1.0)
        Ac = np.ascontiguousarray(
            A.reshape(SB, 128, ND).transpose(1, 0, 2)
        ).astype(NP_FP8)
        xTm = np.ascontiguousarray(xb[ND * c : ND * (c + 1)].T)
        qm = (nodes[ND * c : ND * (c + 1)] < NREAL).astype(f32)
        qmask = np.ascontiguousarray(qm.reshape(NT, 128).T)
        in_maps.append({**shared, "a_cnt": Ac, "xT_mine": xTm, "qmask": qmask})
    return in_maps


def kernel(**inputs):
    nc = _get_nc()
    in_maps = _prep_in_maps(inputs)
    res = run_bass_kernel_spmd(nc, in_maps, core_ids=list(range(NCORES)))
    return np.asarray(res.results[0]["out"], np.float32)


# revision 4
# speedup vs baseline: 1.2193x; 1.1316x over previous
"""GNN message passing + global softmax attention + MLP head on 8 TRN2 NeuronCores.

Strategy (node-sharded SPMD, rank enters only via per-core input data):
  - 2 GraphConv layers: aggregation as block-dense adjacency matmul
    aggT[d, dst] = sum_s x_s^T @ A_s with per-core dense count matrix A
    (fp8, exact small ints) kept SBUF-resident across both layers; fp8
    DoubleRow matmuls (2 src blocks per pass).  h is cast to fp8 for the
    inter-layer AllGather (tiled layout).
  - K = Wk^T ne computed locally, then one AllGather of K (bf16) feeds the
    score loop directly; ne in node-tiled layout gathered off the critical
    path for the final w^T ne matvec.
  - Attention exploits the final mean-pool over queries:
      mean_q softmax(S)_q V = (1/N) (sum_q P[q,:]) V,  w = sum_q P[q,:]
      w^T V = (w^T ne) Wv + (sum w) * vgv.
    Scores S[q,k] on PE, exp on ACT with per-partition shift bias and free
    accum_out giving row sums Z, DVE mult+add accumulates P = E/Z into Pacc
    (bf16).  A ones-matmul reduces Pacc over q into w, matvec w^T ne gives a
    [1,128] row, transpose, one AllReduce, then Wv + MLP head (fp32).
"""

import math

import numpy as np

import concourse.bass as bass
import concourse.bacc as bacc
import concourse.tile as tile
from concourse import mybir
from concourse.bass_utils import run_bass_kernel_spmd
from concourse.masks import make_identity

NCORES = 8
NREAL = 10000
NP = 10240           # padded node count
ND = NP // NCORES    # 1280 nodes per core
NT = ND // 128       # 10 q/dst tiles per core
SB = NP // 128       # 80 src blocks
D = 128
PW = 2048            # exp/score pair width (4 PSUM banks)
NPAIR = NP // PW     # 5 score pairs per q tile
INV = 1.0 / math.sqrt(128.0)
MARGIN = 40.0        # safety margin (scaled units) on the subsample max
PAIR_LAST = NREAL - (NPAIR - 1) * PW  # 1808 valid cols in last pair

BF16 = mybir.dt.bfloat16
FP8 = mybir.dt.float8e4
F32 = mybir.dt.float32

NP_BF16 = mybir.dt.np(BF16)
NP_FP8 = mybir.dt.np(FP8)

_NC_CACHE = {}

RELU = mybir.ActivationFunctionType.Relu
IDENT = mybir.ActivationFunctionType.Identity
EXP = mybir.ActivationFunctionType.Exp
ADD = mybir.AluOpType.add
MULT = mybir.AluOpType.mult
AXX = mybir.AxisListType.X
DR = mybir.MatmulPerfMode.DoubleRow
SL3 = ((0, 512), (512, 1024), (1024, 1280))


def _build():
    nc = bacc.Bacc("TRN2", target_bir_lowering=False, debug=False, num_devices=NCORES)

    A_in = nc.dram_tensor("a_cnt", [128, SB, ND], FP8, kind="ExternalInput")
    xt_in = nc.dram_tensor("x_tiled", [128, SB, D], FP8, kind="ExternalInput")
    xTm_in = nc.dram_tensor("xT_mine", [128, ND], BF16, kind="ExternalInput")
    qmask_in = nc.dram_tensor("qmask", [128, NT], F32, kind="ExternalInput")
    names_bf = ["w1r", "w1l", "w2r", "w2l", "wq", "wk"]
    ins_bf = {n: nc.dram_tensor(n, [D, D], BF16, kind="ExternalInput") for n in names_bf}
    ins_f32 = {
        "b1": nc.dram_tensor("b1", [D, 1], F32, kind="ExternalInput"),
        "b2": nc.dram_tensor("b2", [D, 1], F32, kind="ExternalInput"),
        "qgv": nc.dram_tensor("qgv", [D, 1], F32, kind="ExternalInput"),
        "vgv": nc.dram_tensor("vgv", [D, 1], F32, kind="ExternalInput"),
        "wv32": nc.dram_tensor("wv32", [D, D], F32, kind="ExternalInput"),
        "wo": nc.dram_tensor("wo", [D, D], F32, kind="ExternalInput"),
        "wf1": nc.dram_tensor("wf1", [D, 64], F32, kind="ExternalInput"),
        "wf2": nc.dram_tensor("wf2", [64, 32], F32, kind="ExternalInput"),
        "wf3": nc.dram_tensor("wf3", [32, D], F32, kind="ExternalInput"),
        "bo": nc.dram_tensor("bo", [D, 1], F32, kind="ExternalInput"),
        "bf1": nc.dram_tensor("bf1", [64, 1], F32, kind="ExternalInput"),
        "bf2": nc.dram_tensor("bf2", [32, 1], F32, kind="ExternalInput"),
        "bf3": nc.dram_tensor("bf3", [D, 1], F32, kind="ExternalInput"),
    }
    out_t = nc.dram_tensor("out", [1, D], F32, kind="ExternalOutput")
    rg = [list(range(NCORES))]

    with tile.TileContext(nc) as tc:
        with (
            tc.tile_pool(name="dram", bufs=1, space="DRAM") as dram,
            tc.tile_pool(name="const", bufs=1) as cp,
            tc.tile_pool(name="live", bufs=1) as lp,
        ):
            hb8_a = dram.tile([128, ND // 2], FP8)
            hb8_b = dram.tile([128, ND // 2], FP8)
            h8full_a = dram.tile([NCORES, 128, NT // 2, D], FP8, addr_space="Shared")
            h8full_b = dram.tile([NCORES, 128, NT // 2, D], FP8, addr_space="Shared")
            kb = dram.tile([128, ND], BF16)
            kfull = dram.tile([NCORES, 128, ND], BF16, addr_space="Shared")
            nbt = dram.tile([128, ND], BF16)
            ntfull = dram.tile([NCORES, 128, ND], BF16, addr_space="Shared")
            accb = dram.tile([128, 1], F32)
            accr = dram.tile([128, 1], F32, addr_space="Shared")

            hT = lp.tile([128, ND], BF16)
            QT = lp.tile([128, ND], BF16)
            neT = lp.tile([128, ND], BF16)

            # ================= message passing =================
            with (
                tc.tile_pool(name="sbL", bufs=1) as sl,
                tc.tile_pool(name="psL", bufs=1, space="PSUM") as psl,
            ):
                # input loads: first blocks of x/A first so PE can start early
                xt8 = sl.tile([128, SB, D], FP8)
                asb = sl.tile([128, SB, ND], FP8)
                nc.sync.dma_start(out=xt8[:, 0:20, :], in_=xt_in[:, 0:20, :])
                for s in range(10):
                    nc.sync.dma_start(out=asb[:, s : s + 1, :],
                                      in_=A_in[:, s : s + 1, :])
                for j in range(1, 4):
                    nc.sync.dma_start(out=xt8[:, 20 * j : 20 * (j + 1), :],
                                      in_=xt_in[:, 20 * j : 20 * (j + 1), :])
                for j in range(2, 16):
                    nc.sync.dma_start(
                        out=asb[:, 5 * j : 5 * (j + 1), :],
                        in_=A_in[:, 5 * j : 5 * (j + 1), :],
                    )

                def cload(dram_t, shape, dtype):
                    t = cp.tile(shape, dtype, tag=f"c_{dram_t.name}")
                    nc.sync.dma_start(out=t[:], in_=dram_t[:])
                    return t

                w1r = cload(ins_bf["w1r"], [D, D], BF16)
                w1l = cload(ins_bf["w1l"], [D, D], BF16)
                w2r = cload(ins_bf["w2r"], [D, D], BF16)
                w2l = cload(ins_bf["w2l"], [D, D], BF16)
                wq = cload(ins_bf["wq"], [D, D], BF16)
                wk = cload(ins_bf["wk"], [D, D], BF16)
                b1 = cload(ins_f32["b1"], [D, 1], F32)
                b2 = cload(ins_f32["b2"], [D, 1], F32)
                qgv = cload(ins_f32["qgv"], [D, 1], F32)
                vgv = cload(ins_f32["vgv"], [D, 1], F32)
                wv32 = cload(ins_f32["wv32"], [D, D], F32)
                wo = cload(ins_f32["wo"], [D, D], F32)
                wf1 = cload(ins_f32["wf1"], [D, 64], F32)
                wf2 = cload(ins_f32["wf2"], [64, 32], F32)
                wf3 = cload(ins_f32["wf3"], [32, D], F32)
                bo = cload(ins_f32["bo"], [D, 1], F32)
                bf1 = cload(ins_f32["bf1"], [64, 1], F32)
                bf2 = cload(ins_f32["bf2"], [32, 1], F32)
                bf3 = cload(ins_f32["bf3"], [D, 1], F32)
                xTm = cload(xTm_in, [128, ND], BF16)
                qmask = cload(qmask_in, [128, NT], F32)
                ident = cp.tile([128, 128], F32)
                make_identity(nc, ident[:])
                ones_bf = cp.tile([128, 1], BF16)
                nc.gpsimd.memset(ones_bf[:], 1.0)
                nmarg = cp.tile([128, 1], F32)
                nc.gpsimd.memset(nmarg[:], -MARGIN)

                # ----- layer 1 (fp8 DoubleRow over src-block pairs) -----
                agg_ps = psl.tile([128, ND], F32, tag="agg", space="PSUM")
                for sp in range(SB // 2):
                    for lo, hi in SL3:
                        nc.tensor.matmul(
                            out=agg_ps[:, lo:hi],
                            lhsT=xt8[:, 2 * sp : 2 * sp + 2, :],
                            rhs=asb[:, 2 * sp : 2 * sp + 2, lo:hi],
                            start=(sp == 0), stop=(sp == SB // 2 - 1),
                            perf_mode=DR,
                        )
                agg1 = sl.tile([128, ND], BF16, tag="agg1")
                for lo, hi in SL3:
                    nc.vector.tensor_copy(out=agg1[:, lo:hi], in_=agg_ps[:, lo:hi])

                z_ps = psl.tile([128, ND], F32, tag="z", space="PSUM")
                for lo, hi in SL3:
                    nc.tensor.matmul(out=z_ps[:, lo:hi], lhsT=w1r[:],
                                     rhs=xTm[:, lo:hi], start=True, stop=False)
                    nc.tensor.matmul(out=z_ps[:, lo:hi], lhsT=w1l[:],
                                     rhs=agg1[:, lo:hi], start=False, stop=True)
                    nc.scalar.activation(hT[:, lo:hi], z_ps[:, lo:hi], RELU, bias=b1[:])

                htiled = sl.tile([128, NT, D], BF16, tag="htiled")
                nc.sync.dma_start_transpose(out=htiled[:], in_=hT[:])
                htiled8 = sl.tile([128, NT, D], FP8, tag="htiled8")
                nc.vector.tensor_copy(out=htiled8[:], in_=htiled[:])
                nc.sync.dma_start(
                    out=hb8_a[:],
                    in_=htiled8[:, 0 : NT // 2, :].rearrange("p t d -> p (t d)"),
                )
                nc.sync.dma_start(
                    out=hb8_b[:],
                    in_=htiled8[:, NT // 2 :, :].rearrange("p t d -> p (t d)"),
                )
                nc.gpsimd.collective_compute(
                    "AllGather", mybir.AluOpType.bypass, replica_groups=rg,
                    ins=[hb8_a[:].opt()], outs=[h8full_a[:].opt()],
                )
                nc.gpsimd.collective_compute(
                    "AllGather", mybir.AluOpType.bypass, replica_groups=rg,
                    ins=[hb8_b[:].opt()], outs=[h8full_b[:].opt()],
                )

                # ----- layer 2 (fp8 DoubleRow over tile pairs per core) -----
                hts8 = sl.tile([128, NCORES, NT, D], FP8, tag="hts8")
                for c in range(NCORES):
                    nc.sync.dma_start(out=hts8[:, c, 0 : NT // 2, :],
                                      in_=h8full_a[c])
                for c in range(NCORES):
                    nc.sync.dma_start(out=hts8[:, c, NT // 2 :, :],
                                      in_=h8full_b[c])
                agg_ps2 = psl.tile([128, ND], F32, tag="agg", space="PSUM")
                # t-pairs (0,1),(2,3) only need the first gathered half
                pair_order = [(c, tp) for tp in (0, 1) for c in range(NCORES)]
                pair_order += [(c, tp) for tp in (2, 3, 4) for c in range(NCORES)]
                for pi, (c, tp) in enumerate(pair_order):
                    s = c * NT + 2 * tp
                    for lo, hi in SL3:
                        nc.tensor.matmul(
                            out=agg_ps2[:, lo:hi],
                            lhsT=hts8[:, c, 2 * tp : 2 * tp + 2, :],
                            rhs=asb[:, s : s + 2, lo:hi],
                            start=(pi == 0), stop=(pi == len(pair_order) - 1),
                            perf_mode=DR,
                        )
                agg2 = sl.tile([128, ND], BF16, tag="agg2")
                for lo, hi in SL3:
                    nc.vector.tensor_copy(out=agg2[:, lo:hi], in_=agg_ps2[:, lo:hi])

                z_ps2 = psl.tile([128, ND], F32, tag="z", space="PSUM")
                for lo, hi in SL3:
                    nc.tensor.matmul(out=z_ps2[:, lo:hi], lhsT=w2r[:],
                                     rhs=hT[:, lo:hi], start=True, stop=False)
                    nc.tensor.matmul(out=z_ps2[:, lo:hi], lhsT=w2l[:],
                                     rhs=agg2[:, lo:hi], start=False, stop=True)
                    nc.scalar.activation(neT[:, lo:hi], z_ps2[:, lo:hi], IDENT,
                                         bias=b2[:])

                # K = Wk^T ne locally, gather K (critical path to scores)
                k_ps = psl.tile([128, ND], F32, tag="agg", space="PSUM")
                KTloc = sl.tile([128, ND], BF16, tag="ktloc")
                for lo, hi in SL3:
                    nc.tensor.matmul(out=k_ps[:, lo:hi], lhsT=wk[:],
                                     rhs=neT[:, lo:hi], start=True, stop=True)
                    nc.vector.tensor_copy(out=KTloc[:, lo:hi], in_=k_ps[:, lo:hi])
                for j in range(4):
                    nc.sync.dma_start(out=kb[:, 320 * j : 320 * (j + 1)],
                                      in_=KTloc[:, 320 * j : 320 * (j + 1)])
                nc.gpsimd.collective_compute(
                    "AllGather", mybir.AluOpType.bypass, replica_groups=rg,
                    ins=[kb[:].opt()], outs=[kfull[:].opt()],
                )

                # ne in node-tiled layout, gathered off the critical path
                ntl = sl.tile([128, NT, D], BF16, tag="ntl")
                nc.sync.dma_start_transpose(out=ntl[:], in_=neT[:])
                for j in range(4):
                    nc.sync.dma_start(
                        out=nbt[:, 320 * j : 320 * (j + 1)],
                        in_=ntl[:].rearrange("p t d -> p (t d)")[
                            :, 320 * j : 320 * (j + 1)
                        ],
                    )
                nc.gpsimd.collective_compute(
                    "AllGather", mybir.AluOpType.bypass, replica_groups=rg,
                    ins=[nbt[:].opt()], outs=[ntfull[:].opt()],
                )

                # Q = Wq^T ne + qgv (local; needed a bit later than K)
                q_ps = psl.tile([128, ND], F32, tag="z", space="PSUM")
                for lo, hi in SL3:
                    nc.tensor.matmul(out=q_ps[:, lo:hi], lhsT=wq[:],
                                     rhs=neT[:, lo:hi], start=True, stop=True)
                    nc.vector.tensor_scalar(
                        out=QT[:, lo:hi], in0=q_ps[:, lo:hi],
                        scalar1=qgv[:], scalar2=None, op0=ADD,
                    )

            # ================= attention =================
            with (
                tc.tile_pool(name="sbA", bufs=1) as sa,
                tc.tile_pool(name="etp", bufs=2) as etp,
                tc.tile_pool(name="stp", bufs=2, space="PSUM") as stp,
            ):
                KT = sa.tile([128, NP], BF16)
                for r in range(NCORES):
                    for j in range(4):
                        nc.sync.dma_start(
                            out=KT[:, ND * r + 320 * j : ND * r + 320 * (j + 1)],
                            in_=kfull[r][:, 320 * j : 320 * (j + 1)],
                        )
                net_t = sa.tile([128, SB, D], BF16)
                ntv = net_t[:].rearrange("p s d -> p (s d)")
                for r in range(NCORES):
                    for j in range(2):
                        nc.sync.dma_start(
                            out=ntv[:, ND * r + 640 * j : ND * r + 640 * (j + 1)],
                            in_=ntfull[r][:, 640 * j : 640 * (j + 1)],
                        )

                # per-row shift from stride-16 subsample max (+margin)
                KTs = sa.tile([128, 640], BF16)
                nc.vector.tensor_copy(out=KTs[:], in_=KT[:, ::16])
                negc = sa.tile([128, NT], F32)
                for t in range(NT):
                    sps = stp.tile([128, PW], F32, tag="st", space="PSUM")
                    nc.tensor.matmul(out=sps[:, 0:512],
                                     lhsT=QT[:, 128 * t : 128 * (t + 1)],
                                     rhs=KTs[:, 0:512], start=True, stop=True)
                    nc.tensor.matmul(out=sps[:, 512:640],
                                     lhsT=QT[:, 128 * t : 128 * (t + 1)],
                                     rhs=KTs[:, 512:640], start=True, stop=True)
                    mh = sa.tile([128, 1], F32, tag="mh")
                    nc.vector.reduce_max(mh[:], sps[:, 0:640], axis=AXX)
                    nc.scalar.activation(negc[:, t : t + 1], mh[:], IDENT,
                                         scale=-INV, bias=nmarg[:])

                # main loop: scores -> exp(+Z) -> P accumulation
                Zbuf = sa.tile([128, NT, NPAIR], F32)
                Pacc = [sa.tile([128, PW], BF16, tag=f"pacc{pp}",
                                name=f"pacc{pp}")
                        for pp in range(NPAIR)]
                for t in range(NT):
                    ets = []
                    for pp in range(NPAIR):
                        stt = stp.tile([128, PW], F32, tag="st", space="PSUM")
                        for j in range(4):
                            nc.tensor.matmul(
                                out=stt[:, 512 * j : 512 * (j + 1)],
                                lhsT=QT[:, 128 * t : 128 * (t + 1)],
                                rhs=KT[:, PW * pp + 512 * j : PW * pp + 512 * (j + 1)],
                                start=True, stop=True,
                            )
                        Et = etp.tile([128, PW], BF16, tag=f"e{pp}")
                        if pp < NPAIR - 1:
                            nc.scalar.activation(
                                Et[:], stt[:], EXP, scale=INV,
                                bias=negc[:, t : t + 1],
                                accum_out=Zbuf[:, t, pp : pp + 1],
                            )
                        else:
                            nc.scalar.activation(
                                Et[:], stt[:], EXP, scale=INV,
                                bias=negc[:, t : t + 1],
                            )
                            nc.gpsimd.memset(Et[:, PAIR_LAST:], 0)
                            nc.vector.reduce_sum(Zbuf[:, t, pp : pp + 1],
                                                 Et[:], axis=AXX)
                        ets.append(Et)
                    zt = sa.tile([128, 1], F32, tag=f"zt{t % 2}")
                    nc.vector.reduce_sum(zt[:], Zbuf[:, t, :], axis=AXX)
                    rz = sa.tile([128, 1], F32, tag=f"rz{t % 2}")
                    nc.vector.reciprocal(rz[:], zt[:])
                    rzm = sa.tile([128, 1], F32, tag=f"rzm{t % 2}")
                    nc.vector.tensor_tensor(out=rzm[:], in0=rz[:],
                                            in1=qmask[:, t : t + 1], op=MULT)
                    for pp in range(NPAIR):
                        if t == 0:
                            nc.vector.tensor_scalar(
                                out=Pacc[pp][:], in0=ets[pp][:],
                                scalar1=rzm[:], scalar2=None, op0=MULT,
                            )
                        else:
                            pt = etp.tile([128, PW], BF16, tag="pt")
                            nc.vector.tensor_scalar(
                                out=pt[:], in0=ets[pp][:],
                                scalar1=rzm[:], scalar2=None, op0=MULT,
                            )
                            nc.vector.tensor_tensor(
                                out=Pacc[pp][:], in0=Pacc[pp][:], in1=pt[:],
                                op=ADD,
                            )

                # w[k] = sum_q Pacc[q, k]; out_row = w^T ne, pair-pipelined
                w_ps = stp.tile([128, PW], F32, tag="st", space="PSUM")
                oc_ps = stp.tile([128, PW], F32, tag="st", space="PSUM")
                wsb = sa.tile([128, SB], BF16)

                def w_mms(pp):
                    for j in range(PW // 128):
                        idx = (PW // 128) * pp + j
                        nc.tensor.matmul(
                            out=w_ps[:, idx : idx + 1],
                            lhsT=Pacc[pp][:, 128 * j : 128 * (j + 1)],
                            rhs=ones_bf[:], start=True, stop=True,
                        )
                    nc.vector.tensor_copy(
                        out=wsb[:, 16 * pp : 16 * (pp + 1)],
                        in_=w_ps[:, 16 * pp : 16 * (pp + 1)],
                    )

                def oc_mms(pp):
                    for ci in range(16):
                        c = 16 * pp + ci
                        nc.tensor.matmul(
                            out=oc_ps[:1, 0:128], lhsT=wsb[:, c : c + 1],
                            rhs=net_t[:, c, :],
                            start=(c == 0), stop=(c == SB - 1),
                        )

                w_mms(0)
                for pp in range(1, NPAIR):
                    w_mms(pp)
                    oc_mms(pp - 1)
                oc_mms(NPAIR - 1)

                ocrow = sa.tile([1, 128], F32, tag="ocrow")
                nc.vector.tensor_copy(out=ocrow[:], in_=oc_ps[:1, 0:128])
                tps = stp.tile([128, PW], F32, tag="st", space="PSUM")
                nc.tensor.transpose(out=tps[:, 0:1], in_=ocrow[:],
                                    identity=ident[:1, 0:1])
                accsb = sa.tile([128, 1], F32, tag="accsb")
                nc.vector.tensor_copy(out=accsb[:], in_=tps[:, 0:1])
                nc.sync.dma_start(out=accb[:], in_=accsb[:])
                nc.gpsimd.collective_compute(
                    "AllReduce", ADD, replica_groups=rg,
                    ins=[accb[:].opt()], outs=[accr[:].opt()],
                )
                accg = sa.tile([128, 1], F32, tag="accg")
                nc.sync.dma_start(out=accg[:], in_=accr[:])

                # aggregated = (1/N) Wv^T out_col + vgv, then MLP head
                vps = stp.tile([128, PW], F32, tag="st", space="PSUM")
                nc.tensor.matmul(out=vps[:, 0:1], lhsT=wv32[:], rhs=accg[:],
                                 start=True, stop=True)
                vagg = sa.tile([128, 1], F32, tag="vagg")
                nc.scalar.activation(vagg[:], vps[:, 0:1], IDENT,
                                     scale=1.0 / NREAL, bias=vgv[:])

                hps = stp.tile([128, PW], F32, tag="st", space="PSUM")
                nc.tensor.matmul(out=hps[:, 0:1], lhsT=wo[:], rhs=vagg[:],
                                 start=True, stop=True)
                state = sa.tile([128, 1], F32, tag="state")
                nc.scalar.activation(state[:], hps[:, 0:1], IDENT, bias=bo[:])
                hps2 = stp.tile([128, PW], F32, tag="st", space="PSUM")
                nc.tensor.matmul(out=hps2[:64, 0:1], lhsT=wf1[:], rhs=state[:],
                                 start=True, stop=True)
                x1 = sa.tile([64, 1], F32, tag="x1")
                nc.scalar.activation(x1[:], hps2[:64, 0:1], RELU, bias=bf1[:])
                hps3 = stp.tile([128, PW], F32, tag="st", space="PSUM")
                nc.tensor.matmul(out=hps3[:32, 0:1], lhsT=wf2[:], rhs=x1[:],
                                 start=True, stop=True)
                x2 = sa.tile([32, 1], F32, tag="x2")
                nc.scalar.activation(x2[:], hps3[:32, 0:1], RELU, bias=bf2[:])
                hps4 = stp.tile([128, PW], F32, tag="st", space="PSUM")
                nc.tensor.matmul(out=hps4[:, 0:1], lhsT=wf3[:], rhs=x2[:],
                                 start=True, stop=True)
                lg = sa.tile([128, 1], F32, tag="lg")
                nc.scalar.activation(lg[:], hps4[:, 0:1], IDENT, bias=bf3[:])

                tps2 = stp.tile([128, PW], F32, tag="st", space="PSUM")
                nc.tensor.transpose(out=tps2[:1, 0:128], in_=lg[:],
                                    identity=ident[:])
                er = sa.tile([1, 128], F32, tag="er")
                zf = sa.tile([1, 1], F32, tag="zf")
                nc.scalar.activation(er[:], tps2[:1, 0:128], EXP, accum_out=zf[:])
                rzf = sa.tile([1, 1], F32, tag="rzf")
                nc.vector.reciprocal(rzf[:], zf[:])
                orow = sa.tile([1, 128], F32, tag="orow")
                nc.vector.tensor_scalar(out=orow[:], in0=er[:], scalar1=rzf[:],
                                        scalar2=None, op0=MULT)
                nc.sync.dma_start(out=out_t[:], in_=orow[:])

    nc.compile()
    return nc


def _get_nc():
    if "nc" not in _NC_CACHE:
        _NC_CACHE["nc"] = _build()
    return _NC_CACHE["nc"]


def _prep_in_maps(inputs):
    f32 = np.float32
    x = np.asarray(inputs["node_features"], f32)
    g = np.asarray(inputs["global_info"], f32)
    ei = np.asarray(inputs["edge_index"])
    src = np.asarray(ei[0], np.int64)
    dst = np.asarray(ei[1], np.int64)

    xp = np.zeros((NP, D), f32)
    xp[:NREAL] = x
    xb = xp.astype(NP_BF16)
    x_tiled = np.ascontiguousarray(
        xp.reshape(SB, 128, D).transpose(1, 0, 2)
    ).astype(NP_FP8)

    qgv = (np.asarray(inputs["bQ"], f32)
           + (g @ np.asarray(inputs["WQg"], f32))[0]
           + np.asarray(inputs["bQg"], f32)).reshape(D, 1)
    vgv = (np.asarray(inputs["bV"], f32)
           + (g @ np.asarray(inputs["WVg"], f32))[0]
           + np.asarray(inputs["bVg"], f32)).reshape(D, 1)

    def bf(name):
        return np.ascontiguousarray(np.asarray(inputs[name], f32).astype(NP_BF16))

    shared = {
        "w1r": bf("W1_root"), "w1l": bf("W1_rel"),
        "w2r": bf("W2_root"), "w2l": bf("W2_rel"),
        "wq": bf("WQ"), "wk": bf("WK"),
        "b1": np.asarray(inputs["b1"], f32).reshape(D, 1),
        "b2": np.asarray(inputs["b2"], f32).reshape(D, 1),
        "qgv": qgv, "vgv": vgv,
        "wv32": np.asarray(inputs["WV"], f32),
        "wo": np.asarray(inputs["Wo"], f32),
        "wf1": np.asarray(inputs["Wfc1"], f32),
        "wf2": np.asarray(inputs["Wfc2"], f32),
        "wf3": np.asarray(inputs["Wfc3"], f32),
        "bo": np.asarray(inputs["bo"], f32).reshape(D, 1),
        "bf1": np.asarray(inputs["bfc1"], f32).reshape(64, 1),
        "bf2": np.asarray(inputs["bfc2"], f32).reshape(32, 1),
        "bf3": np.asarray(inputs["bfc3"], f32).reshape(D, 1),
        "x_tiled": x_tiled,
    }

    core_of = dst // ND
    in_maps = []
    nodes = np.arange(NP)
    for c in range(NCORES):
        m = core_of == c
        A = np.zeros((NP, ND), f32)
        np.add.at(A, (src[m], dst[m] - ND * c), 1.0)
        Ac = np.ascontiguousarray(
            A.reshape(SB, 128, ND).transpose(1, 0, 2)
        ).astype(NP_FP8)
        xTm = np.ascontiguousarray(xb[ND * c : ND * (c + 1)].T)
        qm = (nodes[ND * c : ND * (c + 1)] < NREAL).astype(f32)
        qmask = np.ascontiguousarray(qm.reshape(NT, 128).T)
        in_maps.append({**shared, "a_cnt": Ac, "xT_mine": xTm, "qmask": qmask})
    return in_maps


def kernel(**inputs):
    nc = _get_nc()
    in_maps = _prep_in_maps(inputs)
    res = run_bass_kernel_spmd(nc, in_maps, core_ids=list(range(NCORES)))
    return np.asarray(res.results[0]["out"], np.float32)


# revision 7
# speedup vs baseline: 1.2305x; 1.0092x over previous
"""GNN message passing + global softmax attention + MLP head on 8 TRN2 NeuronCores.

Strategy (node-sharded SPMD, rank enters only via per-core input data):
  - 2 GraphConv layers: aggregation as block-dense adjacency matmul
    aggT[d, dst] = sum_s x_s^T @ A_s with per-core dense count matrix A
    (fp8, exact small ints) kept SBUF-resident across both layers; fp8
    DoubleRow matmuls (2 src blocks per pass).  h is cast to fp8 for the
    inter-layer AllGather (tiled layout).
  - K = Wk^T ne computed locally, then one AllGather of K (bf16) feeds the
    score loop directly; ne in node-tiled layout gathered off the critical
    path for the final w^T ne matvec.
  - Attention exploits the final mean-pool over queries:
      mean_q softmax(S)_q V = (1/N) (sum_q P[q,:]) V,  w = sum_q P[q,:]
      w^T V = (w^T ne) Wv + (sum w) * vgv.
    Scores S[q,k] on PE, exp on ACT with per-partition shift bias and free
    accum_out giving row sums Z, DVE mult+add accumulates P = E/Z into Pacc
    (bf16).  A ones-matmul reduces Pacc over q into w, matvec w^T ne gives a
    [1,128] row, transpose, one AllReduce, then Wv + MLP head (fp32).
"""

import math

import numpy as np

import concourse.bass as bass
import concourse.bacc as bacc
import concourse.tile as tile
from concourse import mybir
from concourse.bass_utils import run_bass_kernel_spmd
from concourse.masks import make_identity

NCORES = 8
NREAL = 10000
NP = 10240           # padded node count
ND = NP // NCORES    # 1280 nodes per core
NT = ND // 128       # 10 q/dst tiles per core
SB = NP // 128       # 80 src blocks
D = 128
PW = 2048            # exp/score pair width (4 PSUM banks)
NPAIR = NP // PW     # 5 score pairs per q tile
INV = 1.0 / math.sqrt(128.0)
MARGIN = 40.0        # safety margin (scaled units) on the subsample max
PAIR_LAST = NREAL - (NPAIR - 1) * PW  # 1808 valid cols in last pair

BF16 = mybir.dt.bfloat16
FP8 = mybir.dt.float8e4
F32 = mybir.dt.float32

NP_BF16 = mybir.dt.np(BF16)
NP_FP8 = mybir.dt.np(FP8)

_NC_CACHE = {}

RELU = mybir.ActivationFunctionType.Relu
IDENT = mybir.ActivationFunctionType.Identity
EXP = mybir.ActivationFunctionType.Exp
ADD = mybir.AluOpType.add
MULT = mybir.AluOpType.mult
AXX = mybir.AxisListType.X
DR = mybir.MatmulPerfMode.DoubleRow
SL3 = ((0, 512), (512, 1024), (1024, 1280))


def _build():
    nc = bacc.Bacc("TRN2", target_bir_lowering=False, debug=False, num_devices=NCORES)

    A_in = nc.dram_tensor("a_cnt", [128, SB, ND], FP8, kind="ExternalInput")
    xt_in = nc.dram_tensor("x_tiled", [128, SB, D], FP8, kind="ExternalInput")
    xTm_in = nc.dram_tensor("xT_mine", [128, ND], BF16, kind="ExternalInput")
    qmask_in = nc.dram_tensor("qmask", [128, NT], F32, kind="ExternalInput")
    names_bf = ["w1r", "w1l", "w2r", "w2l", "wq", "wk"]
    ins_bf = {n: nc.dram_tensor(n, [D, D], BF16, kind="ExternalInput") for n in names_bf}
    ins_f32 = {
        "b1": nc.dram_tensor("b1", [D, 1], F32, kind="ExternalInput"),
        "b2": nc.dram_tensor("b2", [D, 1], F32, kind="ExternalInput"),
        "qgv": nc.dram_tensor("qgv", [D, 1], F32, kind="ExternalInput"),
        "vgv": nc.dram_tensor("vgv", [D, 1], F32, kind="ExternalInput"),
        "wv32": nc.dram_tensor("wv32", [D, D], F32, kind="ExternalInput"),
        "wo": nc.dram_tensor("wo", [D, D], F32, kind="ExternalInput"),
        "wf1": nc.dram_tensor("wf1", [D, 64], F32, kind="ExternalInput"),
        "wf2": nc.dram_tensor("wf2", [64, 32], F32, kind="ExternalInput"),
        "wf3": nc.dram_tensor("wf3", [32, D], F32, kind="ExternalInput"),
        "bo": nc.dram_tensor("bo", [D, 1], F32, kind="ExternalInput"),
        "bf1": nc.dram_tensor("bf1", [64, 1], F32, kind="ExternalInput"),
        "bf2": nc.dram_tensor("bf2", [32, 1], F32, kind="ExternalInput"),
        "bf3": nc.dram_tensor("bf3", [D, 1], F32, kind="ExternalInput"),
    }
    out_t = nc.dram_tensor("out", [1, D], F32, kind="ExternalOutput")
    rg = [list(range(NCORES))]

    with tile.TileContext(nc) as tc:
        with (
            tc.tile_pool(name="dram", bufs=1, space="DRAM") as dram,
            tc.tile_pool(name="const", bufs=1) as cp,
            tc.tile_pool(name="live", bufs=1) as lp,
        ):
            hb8_a = dram.tile([128, ND // 2], FP8)
            hb8_b = dram.tile([128, ND // 2], FP8)
            h8full_a = dram.tile([NCORES, 128, NT // 2, D], FP8, addr_space="Shared")
            h8full_b = dram.tile([NCORES, 128, NT // 2, D], FP8, addr_space="Shared")
            kb = dram.tile([128, ND], BF16)
            kfull = dram.tile([NCORES, 128, ND], BF16, addr_space="Shared")
            nbt = dram.tile([128, ND], BF16)
            ntfull = dram.tile([NCORES, 128, ND], BF16, addr_space="Shared")
            accb = dram.tile([128, 1], F32)
            accr = dram.tile([128, 1], F32, addr_space="Shared")

            hT = lp.tile([128, ND], BF16)
            QT = lp.tile([128, ND], BF16)
            neT = lp.tile([128, ND], BF16)

            # ================= message passing =================
            with (
                tc.tile_pool(name="sbL", bufs=1) as sl,
                tc.tile_pool(name="psL", bufs=1, space="PSUM") as psl,
            ):
                # input loads: first blocks of x/A first so PE can start early
                xt8 = sl.tile([128, SB, D], FP8)
                asb = sl.tile([128, SB, ND], FP8)
                nc.sync.dma_start(out=xt8[:, 0:20, :], in_=xt_in[:, 0:20, :])
                for s in range(10):
                    nc.sync.dma_start(out=asb[:, s : s + 1, :],
                                      in_=A_in[:, s : s + 1, :])
                for j in range(1, 4):
                    nc.sync.dma_start(out=xt8[:, 20 * j : 20 * (j + 1), :],
                                      in_=xt_in[:, 20 * j : 20 * (j + 1), :])
                for j in range(2, 16):
                    nc.sync.dma_start(
                        out=asb[:, 5 * j : 5 * (j + 1), :],
                        in_=A_in[:, 5 * j : 5 * (j + 1), :],
                    )

                def cload(dram_t, shape, dtype):
                    t = cp.tile(shape, dtype, tag=f"c_{dram_t.name}")
                    nc.sync.dma_start(out=t[:], in_=dram_t[:])
                    return t

                w1r = cload(ins_bf["w1r"], [D, D], BF16)
                w1l = cload(ins_bf["w1l"], [D, D], BF16)
                w2r = cload(ins_bf["w2r"], [D, D], BF16)
                w2l = cload(ins_bf["w2l"], [D, D], BF16)
                wq = cload(ins_bf["wq"], [D, D], BF16)
                wk = cload(ins_bf["wk"], [D, D], BF16)
                b1 = cload(ins_f32["b1"], [D, 1], F32)
                b2 = cload(ins_f32["b2"], [D, 1], F32)
                qgv = cload(ins_f32["qgv"], [D, 1], F32)
                vgv = cload(ins_f32["vgv"], [D, 1], F32)
                wv32 = cload(ins_f32["wv32"], [D, D], F32)
                wo = cload(ins_f32["wo"], [D, D], F32)
                wf1 = cload(ins_f32["wf1"], [D, 64], F32)
                wf2 = cload(ins_f32["wf2"], [64, 32], F32)
                wf3 = cload(ins_f32["wf3"], [32, D], F32)
                bo = cload(ins_f32["bo"], [D, 1], F32)
                bf1 = cload(ins_f32["bf1"], [64, 1], F32)
                bf2 = cload(ins_f32["bf2"], [32, 1], F32)
                bf3 = cload(ins_f32["bf3"], [D, 1], F32)
                xTm = cload(xTm_in, [128, ND], BF16)
                qmask = cload(qmask_in, [128, NT], F32)
                ident = cp.tile([128, 128], F32)
                make_identity(nc, ident[:])
                ones_bf = cp.tile([128, 1], BF16)
                nc.gpsimd.memset(ones_bf[:], 1.0)
                nmarg = cp.tile([128, 1], F32)
                nc.gpsimd.memset(nmarg[:], -MARGIN)

                # ----- layer 1 (fp8 DoubleRow over src-block pairs) -----
                agg_ps = psl.tile([128, ND], F32, tag="agg", space="PSUM")
                for sp in range(SB // 2):
                    for lo, hi in SL3:
                        nc.tensor.matmul(
                            out=agg_ps[:, lo:hi],
                            lhsT=xt8[:, 2 * sp : 2 * sp + 2, :],
                            rhs=asb[:, 2 * sp : 2 * sp + 2, lo:hi],
                            start=(sp == 0), stop=(sp == SB // 2 - 1),
                            perf_mode=DR,
                        )
                agg1 = sl.tile([128, ND], BF16, tag="agg1")
                for lo, hi in SL3:
                    nc.vector.tensor_copy(out=agg1[:, lo:hi], in_=agg_ps[:, lo:hi])

                z_ps = psl.tile([128, ND], F32, tag="z", space="PSUM")
                for lo, hi in SL3:
                    nc.tensor.matmul(out=z_ps[:, lo:hi], lhsT=w1r[:],
                                     rhs=xTm[:, lo:hi], start=True, stop=False)
                    nc.tensor.matmul(out=z_ps[:, lo:hi], lhsT=w1l[:],
                                     rhs=agg1[:, lo:hi], start=False, stop=True)
                    nc.scalar.activation(hT[:, lo:hi], z_ps[:, lo:hi], RELU, bias=b1[:])

                htiled = sl.tile([128, NT, D], BF16, tag="htiled")
                nc.sync.dma_start_transpose(out=htiled[:], in_=hT[:])
                htiled8 = sl.tile([128, NT, D], FP8, tag="htiled8")
                nc.vector.tensor_copy(out=htiled8[:], in_=htiled[:])
                nc.sync.dma_start(
                    out=hb8_a[:],
                    in_=htiled8[:, 0 : NT // 2, :].rearrange("p t d -> p (t d)"),
                )
                nc.sync.dma_start(
                    out=hb8_b[:],
                    in_=htiled8[:, NT // 2 :, :].rearrange("p t d -> p (t d)"),
                )
                nc.gpsimd.collective_compute(
                    "AllGather", mybir.AluOpType.bypass, replica_groups=rg,
                    ins=[hb8_a[:].opt()], outs=[h8full_a[:].opt()],
                )
                nc.gpsimd.collective_compute(
                    "AllGather", mybir.AluOpType.bypass, replica_groups=rg,
                    ins=[hb8_b[:].opt()], outs=[h8full_b[:].opt()],
                )

                # ----- layer 2 (fp8 DoubleRow over tile pairs per core) -----
                hts8 = sl.tile([128, NCORES, NT, D], FP8, tag="hts8")
                for c in range(NCORES):
                    nc.sync.dma_start(out=hts8[:, c, 0 : NT // 2, :],
                                      in_=h8full_a[c])
                for c in range(NCORES):
                    nc.sync.dma_start(out=hts8[:, c, NT // 2 :, :],
                                      in_=h8full_b[c])
                agg_ps2 = psl.tile([128, ND], F32, tag="agg", space="PSUM")
                # t-pairs (0,1),(2,3) only need the first gathered half
                pair_order = [(c, tp) for tp in (0, 1) for c in range(NCORES)]
                pair_order += [(c, tp) for tp in (2, 3, 4) for c in range(NCORES)]
                for pi, (c, tp) in enumerate(pair_order):
                    s = c * NT + 2 * tp
                    for lo, hi in SL3:
                        nc.tensor.matmul(
                            out=agg_ps2[:, lo:hi],
                            lhsT=hts8[:, c, 2 * tp : 2 * tp + 2, :],
                            rhs=asb[:, s : s + 2, lo:hi],
                            start=(pi == 0), stop=(pi == len(pair_order) - 1),
                            perf_mode=DR,
                        )
                agg2 = sl.tile([128, ND], BF16, tag="agg2")
                for lo, hi in SL3:
                    nc.vector.tensor_copy(out=agg2[:, lo:hi], in_=agg_ps2[:, lo:hi])

                z_ps2 = psl.tile([128, ND], F32, tag="z", space="PSUM")
                for lo, hi in SL3:
                    nc.tensor.matmul(out=z_ps2[:, lo:hi], lhsT=w2r[:],
                                     rhs=hT[:, lo:hi], start=True, stop=False)
                    nc.tensor.matmul(out=z_ps2[:, lo:hi], lhsT=w2l[:],
                                     rhs=agg2[:, lo:hi], start=False, stop=True)
                    nc.scalar.activation(neT[:, lo:hi], z_ps2[:, lo:hi], IDENT,
                                         bias=b2[:])

                # K = Wk^T ne locally, gather K (critical path to scores)
                k_ps = psl.tile([128, ND], F32, tag="agg", space="PSUM")
                KTloc = sl.tile([128, ND], BF16, tag="ktloc")
                for lo, hi in SL3:
                    nc.tensor.matmul(out=k_ps[:, lo:hi], lhsT=wk[:],
                                     rhs=neT[:, lo:hi], start=True, stop=True)
                    nc.vector.tensor_copy(out=KTloc[:, lo:hi], in_=k_ps[:, lo:hi])
                for j in range(4):
                    nc.sync.dma_start(out=kb[:, 320 * j : 320 * (j + 1)],
                                      in_=KTloc[:, 320 * j : 320 * (j + 1)])
                nc.gpsimd.collective_compute(
                    "AllGather", mybir.AluOpType.bypass, replica_groups=rg,
                    ins=[kb[:].opt()], outs=[kfull[:].opt()],
                )

                # ne in node-tiled layout, gathered off the critical path
                ntl = sl.tile([128, NT, D], BF16, tag="ntl")
                nc.sync.dma_start_transpose(out=ntl[:], in_=neT[:])
                for j in range(4):
                    nc.sync.dma_start(
                        out=nbt[:, 320 * j : 320 * (j + 1)],
                        in_=ntl[:].rearrange("p t d -> p (t d)")[
                            :, 320 * j : 320 * (j + 1)
                        ],
                    )
                nc.gpsimd.collective_compute(
                    "AllGather", mybir.AluOpType.bypass, replica_groups=rg,
                    ins=[nbt[:].opt()], outs=[ntfull[:].opt()],
                )

                # Q = Wq^T ne + qgv (local; needed a bit later than K)
                q_ps = psl.tile([128, ND], F32, tag="z", space="PSUM")
                for lo, hi in SL3:
                    nc.tensor.matmul(out=q_ps[:, lo:hi], lhsT=wq[:],
                                     rhs=neT[:, lo:hi], start=True, stop=True)
                    nc.vector.tensor_scalar(
                        out=QT[:, lo:hi], in0=q_ps[:, lo:hi],
                        scalar1=qgv[:], scalar2=None, op0=ADD,
                    )

            # ================= attention =================
            with (
                tc.tile_pool(name="sbA", bufs=1) as sa,
                tc.tile_pool(name="etp", bufs=2) as etp,
                tc.tile_pool(name="stp", bufs=2, space="PSUM") as stp,
            ):
                KT = sa.tile([128, NP], BF16)
                for r in range(NCORES):
                    for j in range(4):
                        nc.sync.dma_start(
                            out=KT[:, ND * r + 320 * j : ND * r + 320 * (j + 1)],
                            in_=kfull[r][:, 320 * j : 320 * (j + 1)],
                        )
                net_t = sa.tile([128, SB, D], BF16)
                ntv = net_t[:].rearrange("p s d -> p (s d)")
                for r in range(NCORES):
                    for j in range(2):
                        nc.sync.dma_start(
                            out=ntv[:, ND * r + 640 * j : ND * r + 640 * (j + 1)],
                            in_=ntfull[r][:, 640 * j : 640 * (j + 1)],
                        )

                # per-row shift from stride-16 subsample max (+margin)
                KTs = sa.tile([128, 640], BF16)
                nc.vector.tensor_copy(out=KTs[:], in_=KT[:, ::16])
                negc = sa.tile([128, NT], F32)
                for t in range(NT):
                    sps = stp.tile([128, PW], F32, tag="st", space="PSUM")
                    nc.tensor.matmul(out=sps[:, 0:512],
                                     lhsT=QT[:, 128 * t : 128 * (t + 1)],
                                     rhs=KTs[:, 0:512], start=True, stop=True)
                    nc.tensor.matmul(out=sps[:, 512:640],
                                     lhsT=QT[:, 128 * t : 128 * (t + 1)],
                                     rhs=KTs[:, 512:640], start=True, stop=True)
                    mh = sa.tile([128, 1], F32, tag="mh")
                    nc.vector.reduce_max(mh[:], sps[:, 0:640], axis=AXX)
                    nc.scalar.activation(negc[:, t : t + 1], mh[:], IDENT,
                                         scale=-INV, bias=nmarg[:])

                # main loop: scores -> exp(+Z) -> P accumulation
                Zbuf = sa.tile([128, NT, NPAIR], F32)
                Pacc = [[sa.tile([128, PW], BF16, tag=f"pacc{pp}_{g}",
                                 name=f"pacc{pp}_{g}")
                         for g in range(2)]
                        for pp in range(NPAIR)]
                for t in range(NT):
                    ets = []
                    for pp in range(NPAIR):
                        stt = stp.tile([128, PW], F32, tag="st", space="PSUM")
                        for j in range(4):
                            nc.tensor.matmul(
                                out=stt[:, 512 * j : 512 * (j + 1)],
                                lhsT=QT[:, 128 * t : 128 * (t + 1)],
                                rhs=KT[:, PW * pp + 512 * j : PW * pp + 512 * (j + 1)],
                                start=True, stop=True,
                            )
                        if pp == NPAIR - 1:
                            # mask padded k columns before exp so the free
                            # accum_out row sum stays exact
                            nc.vector.memset(stt[:, PAIR_LAST:], -1e6)
                        Et = etp.tile([128, PW], BF16, tag=f"e{pp}")
                        nc.scalar.activation(
                            Et[:], stt[:], EXP, scale=INV,
                            bias=negc[:, t : t + 1],
                            accum_out=Zbuf[:, t, pp : pp + 1],
                        )
                        ets.append(Et)
                    zt = sa.tile([128, 1], F32, tag=f"zt{t % 2}")
                    nc.vector.reduce_sum(zt[:], Zbuf[:, t, :], axis=AXX)
                    rz = sa.tile([128, 1], F32, tag=f"rz{t % 2}")
                    nc.vector.reciprocal(rz[:], zt[:])
                    rzm = sa.tile([128, 1], F32, tag=f"rzm{t % 2}")
                    nc.vector.tensor_tensor(out=rzm[:], in0=rz[:],
                                            in1=qmask[:, t : t + 1], op=MULT)
                    for pp in range(NPAIR):
                        if t == 0:
                            nc.vector.tensor_scalar(
                                out=Pacc[pp][0][:], in0=ets[pp][:],
                                scalar1=rzm[:], scalar2=None, op0=MULT,
                            )
                        else:
                            pt = etp.tile([128, PW], BF16, tag="pt")
                            nc.vector.tensor_scalar(
                                out=pt[:], in0=ets[pp][:],
                                scalar1=rzm[:], scalar2=None, op0=MULT,
                            )
                            nc.vector.tensor_tensor(
                                out=Pacc[pp][t % 2][:],
                                in0=Pacc[pp][(t - 1) % 2][:], in1=pt[:],
                                op=ADD,
                            )

                # w[k] = sum_q Pacc[q, k]; out_row = w^T ne, pair-pipelined
                w_ps = stp.tile([128, PW], F32, tag="st", space="PSUM")
                oc_ps = stp.tile([128, PW], F32, tag="st", space="PSUM")
                wsb = sa.tile([128, SB], BF16)

                def w_mms(pp):
                    for j in range(PW // 128):
                        idx = (PW // 128) * pp + j
                        nc.tensor.matmul(
                            out=w_ps[:, idx : idx + 1],
                            lhsT=Pacc[pp][(NT - 1) % 2][:, 128 * j : 128 * (j + 1)],
                            rhs=ones_bf[:], start=True, stop=True,
                        )
                    nc.scalar.copy(
                        out=wsb[:, 16 * pp : 16 * (pp + 1)],
                        in_=w_ps[:, 16 * pp : 16 * (pp + 1)],
                    )

                def oc_mms(pp):
                    for ci in range(16):
                        c = 16 * pp + ci
                        nc.tensor.matmul(
                            out=oc_ps[:1, 0:128], lhsT=wsb[:, c : c + 1],
                            rhs=net_t[:, c, :],
                            start=(c == 0), stop=(c == SB - 1),
                        )

                w_mms(0)
                for pp in range(1, NPAIR):
                    w_mms(pp)
                    oc_mms(pp - 1)
                oc_mms(NPAIR - 1)

                ocrow = sa.tile([1, 128], F32, tag="ocrow")
                nc.scalar.copy(out=ocrow[:], in_=oc_ps[:1, 0:128])
                tps = stp.tile([128, PW], F32, tag="st", space="PSUM")
                nc.tensor.transpose(out=tps[:, 0:1], in_=ocrow[:],
                                    identity=ident[:1, 0:1])
                accsb = sa.tile([128, 1], F32, tag="accsb")
                nc.scalar.copy(out=accsb[:], in_=tps[:, 0:1])
                nc.sync.dma_start(out=accb[:], in_=accsb[:])
                nc.gpsimd.collective_compute(
                    "AllReduce", ADD, replica_groups=rg,
                    ins=[accb[:].opt()], outs=[accr[:].opt()],
                )
                accg = sa.tile([128, 1], F32, tag="accg")
                nc.sync.dma_start(out=accg[:], in_=accr[:])

                # aggregated = (1/N) Wv^T out_col + vgv, then MLP head
                vps = stp.tile([128, PW], F32, tag="st", space="PSUM")
                nc.tensor.matmul(out=vps[:, 0:1], lhsT=wv32[:], rhs=accg[:],
                                 start=True, stop=True)
                vagg = sa.tile([128, 1], F32, tag="vagg")
                nc.scalar.activation(vagg[:], vps[:, 0:1], IDENT,
                                     scale=1.0 / NREAL, bias=vgv[:])

                hps = stp.tile([128, PW], F32, tag="st", space="PSUM")
                nc.tensor.matmul(out=hps[:, 0:1], lhsT=wo[:], rhs=vagg[:],
                                 start=True, stop=True)
                state = sa.tile([128, 1], F32, tag="state")
                nc.scalar.activation(state[:], hps[:, 0:1], IDENT, bias=bo[:])
                hps2 = stp.tile([128, PW], F32, tag="st", space="PSUM")
                nc.tensor.matmul(out=hps2[:64, 0:1], lhsT=wf1[:], rhs=state[:],
                                 start=True, stop=True)
                x1 = sa.tile([64, 1], F32, tag="x1")
                nc.scalar.activation(x1[:], hps2[:64, 0:1], RELU, bias=bf1[:])
                hps3 = stp.tile([128, PW], F32, tag="st", space="PSUM")
                nc.tensor.matmul(out=hps3[:32, 0:1], lhsT=wf2[:], rhs=x1[:],
                                 start=True, stop=True)
                x2 = sa.tile([32, 1], F32, tag="x2")
                nc.scalar.activation(x2[:], hps3[:32, 0:1], RELU, bias=bf2[:])
                hps4 = stp.tile([128, PW], F32, tag="st", space="PSUM")
                nc.tensor.matmul(out=hps4[:, 0:1], lhsT=wf3[:], rhs=x2[:],
                                 start=True, stop=True)
                lg = sa.tile([128, 1], F32, tag="lg")
                nc.scalar.activation(lg[:], hps4[:, 0:1], IDENT, bias=bf3[:])

                tps2 = stp.tile([128, PW], F32, tag="st", space="PSUM")
                nc.tensor.transpose(out=tps2[:1, 0:128], in_=lg[:],
                                    identity=ident[:])
                er = sa.tile([1, 128], F32, tag="er")
                zf = sa.tile([1, 1], F32, tag="zf")
                nc.scalar.activation(er[:], tps2[:1, 0:128], EXP, accum_out=zf[:])
                rzf = sa.tile([1, 1], F32, tag="rzf")
                nc.vector.reciprocal(rzf[:], zf[:])
                orow = sa.tile([1, 128], F32, tag="orow")
                nc.vector.tensor_scalar(out=orow[:], in0=er[:], scalar1=rzf[:],
                                        scalar2=None, op0=MULT)
                nc.sync.dma_start(out=out_t[:], in_=orow[:])

    nc.compile()
    return nc


def _get_nc():
    if "nc" not in _NC_CACHE:
        _NC_CACHE["nc"] = _build()
    return _NC_CACHE["nc"]


def _prep_in_maps(inputs):
    f32 = np.float32
    x = np.asarray(inputs["node_features"], f32)
    g = np.asarray(inputs["global_info"], f32)
    ei = np.asarray(inputs["edge_index"])
    src = np.asarray(ei[0], np.int64)
    dst = np.asarray(ei[1], np.int64)

    xp = np.zeros((NP, D), f32)
    xp[:NREAL] = x
    xb = xp.astype(NP_BF16)
    x_tiled = np.ascontiguousarray(
        xp.reshape(SB, 128, D).transpose(1, 0, 2)
    ).astype(NP_FP8)

    qgv = (np.asarray(inputs["bQ"], f32)
           + (g @ np.asarray(inputs["WQg"], f32))[0]
           + np.asarray(inputs["bQg"], f32)).reshape(D, 1)
    vgv = (np.asarray(inputs["bV"], f32)
           + (g @ np.asarray(inputs["WVg"], f32))[0]
           + np.asarray(inputs["bVg"], f32)).reshape(D, 1)

    def bf(name):
        return np.ascontiguousarray(np.asarray(inputs[name], f32).astype(NP_BF16))

    shared = {
        "w1r": bf("W1_root"), "w1l": bf("W1_rel"),
        "w2r": bf("W2_root"), "w2l": bf("W2_rel"),
        "wq": bf("WQ"), "wk": bf("WK"),
        "b1": np.asarray(inputs["b1"], f32).reshape(D, 1),
        "b2": np.asarray(inputs["b2"], f32).reshape(D, 1),
        "qgv": qgv, "vgv": vgv,
        "wv32": np.asarray(inputs["WV"], f32),
        "wo": np.asarray(inputs["Wo"], f32),
        "wf1": np.asarray(inputs["Wfc1"], f32),
        "wf2": np.asarray(inputs["Wfc2"], f32),
        "wf3": np.asarray(inputs["Wfc3"], f32),
        "bo": np.asarray(inputs["bo"], f32).reshape(D, 1),
        "bf1": np.asarray(inputs["bfc1"], f32).reshape(64, 1),
        "bf2": np.asarray(inputs["bfc2"], f32).reshape(32, 1),
        "bf3": np.asarray(inputs["bfc3"], f32).reshape(D, 1),
        "x_tiled": x_tiled,
    }

    core_of = dst // ND
    in_maps = []
    nodes = np.arange(NP)
    for c in range(NCORES):
        m = core_of == c
        A = np.zeros((NP, ND), f32)
        np.add.at(A, (src[m], dst[m] - ND * c), 1.0)
        Ac = np.ascontiguousarray(
            A.reshape(SB, 128, ND).transpose(1, 0, 2)
        ).astype(NP_FP8)
        xTm = np.ascontiguousarray(xb[ND * c : ND * (c + 1)].T)
        qm = (nodes[ND * c : ND * (c + 1)] < NREAL).astype(f32)
        qmask = np.ascontiguousarray(qm.reshape(NT, 128).T)
        in_maps.append({**shared, "a_cnt": Ac, "xT_mine": xTm, "qmask": qmask})
    return in_maps


def kernel(**inputs):
    nc = _get_nc()
    in_maps = _prep_in_maps(inputs)
    res = run_bass_kernel_spmd(nc, in_maps, core_ids=list(range(NCORES)))
    return np.asarray(res.results[0]["out"], np.float32)
